# revision 25
# baseline (speedup 1.0000x reference)
"""Trainium2 Bass kernel for nn_DiffuserJointer (dense diffusion transformer).

Strategy: data-parallel over batch B=8 across 8 NeuronCores (one batch
element per core; no collectives). On-core layout is feature-major
(features on partitions, tokens on the free dim) so every matmul contracts
over partitions with no transposes. All matmuls run in float32r (fp32 bits,
~tf32 precision, bf16-rate on the PE). Attention uses a transposed-scores
formulation (kv on partitions) so softmax normalization folds into an
augmented ones-column of V; RoPE's pair-shuffle is a constant 128x128
matmul; per-token normalizers broadcast via rank-1 matmuls.
"""
import sys
sys.path.insert(0, '/opt/trn_rl_repo')

import numpy as np

import concourse.bass as bass
import concourse.bacc as bacc
import concourse.mybir as mybir
import concourse.tile as tile
from concourse.bass_utils import run_bass_kernel_spmd

f32 = mybir.dt.float32
f32r = mybir.dt.float32r
bf16 = mybir.dt.bfloat16
AF = mybir.ActivationFunctionType
ALU = mybir.AluOpType

# ---- problem dims (hardcoded) ----
D, H, B, L, N, K, NHIST = 384, 8, 8, 64, 2560, 512, 3
HD = D // H            # 48
DP = 512               # padded q/k feature dim (64 per head)
TS = L + K             # 576 self-attn tokens
NT = 3                 # feature tiles of 128
NTP = 4                # padded feature tiles
N_CORES = 8
EPS = 1e-5

# optional debug taps: list of names filled by the test harness before build
DEBUG_TAPS = []
_TAP_SHAPES = {}


# ---------------------------------------------------------------- host prep
def _sin_emb_table(x, dim):
    half = dim // 2
    freqs = np.exp(-np.log(10000.0) / (half - 1) * np.arange(half, dtype=np.float64))
    ang = np.asarray(x, np.float64)[:, None] * freqs[None, :]
    return np.concatenate([np.sin(ang), np.cos(ang)], axis=-1).astype(np.float32)


def _rope_div():
    d = D // 3  # 128
    return np.exp(np.arange(0, d, 2, dtype=np.float64)
                  * (-np.log(10000.0) / d)).astype(np.float32)


def _pad_rows(w):
    """(384, ...) head rows -> (512, ...): head h rows h*48..+47 -> h*64..+47."""
    out = np.zeros((DP,) + w.shape[1:], dtype=np.float32)
    for h in range(H):
        out[h * 64:h * 64 + HD] = w[h * HD:(h + 1) * HD]
    return out


def _tile_lhsT(wT):
    din = wT.shape[0]
    assert din % 128 == 0
    return np.ascontiguousarray(wT.reshape(din // 128, 128, *wT.shape[1:]))


def _cols(v):
    return np.ascontiguousarray(np.asarray(v, np.float32).reshape(NT, 128).T)


def _cols_p(v):
    return np.ascontiguousarray(np.asarray(v, np.float32).reshape(NTP, 128).T)


def _shuffle_mat(padded):
    S = np.zeros((128, 128), dtype=np.float32)
    if padded:
        for b0 in (0, 64):
            for i in range(0, HD, 2):
                S[b0 + i, b0 + i + 1] = -1.0
                S[b0 + i + 1, b0 + i] = 1.0
    else:
        for i in range(0, 128, 2):
            S[i, i + 1] = -1.0
            S[i + 1, i] = 1.0
    return np.ascontiguousarray(S.T)


def _divmat(padded):
    div = _rope_div()
    n = DP if padded else D
    M = np.zeros((3, n), dtype=np.float32)
    for r in range(n):
        if padded:
            h, j = r // 64, r % 64
            if j >= HD:
                continue
            d = h * HD + j
        else:
            d = r
        M[d // 128, r] = div[(d % 128) // 2]
    return M


def _pad_feat_T(x):
    """(T, 384) -> (512, T) padded feature-major."""
    xT = np.zeros((DP, x.shape[0]), dtype=np.float32)
    for h in range(H):
        xT[h * 64:h * 64 + HD] = x[:, h * HD:(h + 1) * HD].T
    return np.ascontiguousarray(xT)


def prep_weights(params):
    p = {k: (np.asarray(v, np.float32) if not isinstance(v, (list, dict)) else v)
         for k, v in params.items()}
    w = {}

    def blk(name, bp, cross):
        bp = {k: np.asarray(v, np.float32) for k, v in bp.items()}
        wq, wk, wv = bp['in_w'][:D], bp['in_w'][D:2 * D], bp['in_w'][2 * D:]
        bq, bk, bv = bp['in_b'][:D], bp['in_b'][D:2 * D], bp['in_b'][2 * D:]
        sc = HD ** -0.5
        wq_p, bq_p = _pad_rows(wq * sc), _pad_rows(bq * sc)
        w[f'{name}_wq'] = _tile_lhsT(wq_p.T.copy())           # (3,128,512)
        w[f'{name}_wk'] = _tile_lhsT(_pad_rows(wk).T.copy())  # (3,128,512)
        bk_cols = _cols_p(_pad_rows(bk))
        w[f'{name}_wv'] = _tile_lhsT(wv.T.copy())             # (3,128,384)
        w[f'{name}_wo'] = _tile_lhsT(_pad_rows(bp['out_w'].T.copy()))
        w[f'{name}_w1'] = _tile_lhsT(bp['w1'].T.copy())
        w[f'{name}_w2'] = _tile_lhsT(bp['w2'].T.copy())
        ada_cat = np.concatenate([bp['ada_w'], bp['f_ada_w']], axis=0)
        w[f'{name}_ada'] = _tile_lhsT(ada_cat.T.copy())       # (3,128,1536)
        cols = [
            _cols_p(bq_p),                               # 0:4
            bk_cols,                                     # 4:8
            _cols(bp['out_b']),                          # 8:11
            _cols(bp['b1']),                             # 11:14
            _cols(bp['b2']),                             # 14:17
            np.ascontiguousarray(np.concatenate(
                [bp['ada_b'], bp['f_ada_b']]).reshape(12, 128).T),  # 17:29
            _cols(bp['ln1_g']), _cols(bp['ln1_b']),      # 29:32, 32:35
            _cols(bp['ln2_g']), _cols(bp['ln2_b']),      # 35:38, 38:41
        ]
        w[f'{name}_bias'] = np.ascontiguousarray(np.concatenate(cols, axis=1))
        w[f'{name}_bv'] = np.ascontiguousarray(bv.reshape(1, D))
        w[f'{name}_adab'] = np.ascontiguousarray(np.concatenate(
            [bp['ada_b'], bp['f_ada_b']]).reshape(12, 128).T)

    for i, bp in enumerate(p['cross']):
        blk(f'c{i}', bp, cross=True)
    for i, bp in enumerate(p['selfa']):
        blk(f's{i}', bp, cross=False)
    for i, bp in enumerate(p['posa']):
        blk(f'p{i}', bp, cross=False)

    traj_aug = np.concatenate([p['traj_w'], p['traj_b'][:, None]], axis=1)
    w['traj_wT'] = np.ascontiguousarray(traj_aug.T)     # (8, 384)
    w['te1'] = _tile_lhsT(p['te1_w'].T.copy())
    w['te2'] = _tile_lhsT(p['te2_w'].T.copy())
    w['cg1'] = _tile_lhsT(p['cg1_w'].T.copy())          # (9,128,384)
    w['cg2'] = _tile_lhsT(p['cg2_w'].T.copy())
    w['enc_bias'] = np.ascontiguousarray(np.concatenate(
        [_cols(p['te1_b']), _cols(p['te2_b']), _cols(p['cg1_b']),
         _cols(p['cg2_b'])], axis=1))                   # (128, 12)
    w['pproj'] = _tile_lhsT(p['pproj_w'].T.copy())
    w['pp1'] = _tile_lhsT(p['pp1_w'].T.copy())
    w['op1'] = _tile_lhsT(p['op1_w'].T.copy())
    w['head_bias'] = np.ascontiguousarray(np.concatenate(
        [_cols(p['pproj_b']), _cols(p['pp1_b']), _cols(p['op1_b'])], axis=1))
    w['pp2T'] = _tile_lhsT(p['pp2_w'].T.copy())         # (3,128,7)
    w['op2T'] = _tile_lhsT(p['op2_w'].T.copy())         # (3,128,1)
    w['pp2_b'] = np.ascontiguousarray(p['pp2_b'].reshape(1, 7))
    w['op2_b'] = np.ascontiguousarray(p['op2_b'].reshape(1, 1))

    w['S_pad'] = _shuffle_mat(True)
    w['S_nop'] = _shuffle_mat(False)
    w['divmat_p'] = _divmat(True)                        # (3,512)
    w['divmat_n'] = _divmat(False)                       # (3,384)
    w['sinembL'] = _tile_lhsT(np.ascontiguousarray(
        _sin_emb_table(np.arange(L), D).T))              # (3,128,64)
    half = D // 2
    tf = np.exp(-np.log(10000.0) / (half - 1)
                * np.arange(half)).astype(np.float32)
    w['tfreq'] = np.ascontiguousarray(np.concatenate([tf, tf]).reshape(1, D))
    phase = np.zeros(D, np.float32)
    phase[half:] = np.pi / 2
    w['tphase'] = _cols(phase)                           # (128,3)
    return w


def prep_core_inputs(b, inputs):
    m = {}
    traj = np.asarray(inputs['trajectory'], np.float32)[b]          # (64,7)
    m['traj_aug'] = np.ascontiguousarray(
        np.concatenate([traj.T, np.ones((1, L), np.float32)], axis=0))
    m['traj_xyz'] = np.ascontiguousarray(traj[:, :3].T)             # (3,64)
    m['tstep'] = np.array([[float(np.asarray(inputs['timestep'])[b])]],
                          np.float32)
    m['ctxT'] = np.ascontiguousarray(
        np.asarray(inputs['context_feats'], np.float32)[b].T
        .reshape(NT, 128, N))
    m['ctx_xyz'] = np.ascontiguousarray(
        np.asarray(inputs['context'], np.float32)[b].T)             # (3,2560)
    m['gflat'] = np.ascontiguousarray(
        np.asarray(inputs['adaln_gripper_feats'], np.float32)[b]
        .reshape(-1).reshape(9, 128, 1))
    m['fpsT'] = np.ascontiguousarray(
        np.asarray(inputs['fps_feats'], np.float32)[:, b, :].T
        .reshape(NT, 128, K))
    fp = np.asarray(inputs['fps_pos'], np.float32)[b]               # (512,384,2)
    m['fps_cosT'] = np.ascontiguousarray(
        _pad_feat_T(fp[:, :, 0]).reshape(NTP, 128, K))
    m['fps_sinT'] = np.ascontiguousarray(
        _pad_feat_T(fp[:, :, 1]).reshape(NTP, 128, K))
    return m


# ------------------------------------------------------------- bass builder
def _nc_chunks(T):
    if T <= 512:
        return [(0, T)]
    if T == TS:
        return [(0, 288), (288, 288)]
    return [(i, 320) for i in range(0, T, 320)]


def _kv_chunks(T):
    out, i = [], 0
    while i < T:
        c = min(128, T - i)
        out.append((i, c))
        i += c
    return out


def _head_rows(h, padded):
    if padded:
        return [(h // 2, (h % 2) * 64, HD)]
    lo, hi = h * HD, (h + 1) * HD
    out = []
    for t in range(lo // 128, (hi - 1) // 128 + 1):
        s = max(lo, t * 128) - t * 128
        e = min(hi, (t + 1) * 128) - t * 128
        out.append((t, s, e - s))
    return out


class _BD:
    pass


def build(debug_taps=()):
    bd = _BD()
    nc = bacc.Bacc('TRN2', target_bir_lowering=False, debug=False,
                   num_devices=N_CORES)
    bd.nc = nc
    dram = {}

    def din(name, shape, dtype=f32r):
        dram[name] = nc.dram_tensor(name, list(shape), dtype,
                                    kind='ExternalInput').ap()

    def dout(name, shape, dtype=f32):
        dram[name] = nc.dram_tensor(name, list(shape), dtype,
                                    kind='ExternalOutput').ap()

    din('traj_aug', (8, L))
    din('traj_xyz', (3, L))
    din('tstep', (1, 1))
    din('ctxT', (NT, 128, N))
    din('ctx_xyz', (3, N))
    din('gflat', (9, 128, 1))
    din('fpsT', (NT, 128, K))
    din('fps_cosT', (NTP, 128, K))
    din('fps_sinT', (NTP, 128, K))
    blocks = ['c0', 'c1', 's0', 's1', 's2', 's3', 'p0', 'p1']
    for nm in blocks:
        cross = nm.startswith('c')
        din(f'{nm}_wq', (NT, 128, DP))
        din(f'{nm}_wk', (NT, 128, DP))
        din(f'{nm}_wv', (NT, 128, D))
        din(f'{nm}_wo', (NTP, 128, D))
        din(f'{nm}_w1', (NT, 128, D))
        din(f'{nm}_w2', (NT, 128, D))
        din(f'{nm}_ada', (NT, 128, 4 * D))
        din(f'{nm}_bias', (128, 41), f32)
        din(f'{nm}_adab', (128, 12), f32)
        din(f'{nm}_bv', (1, D))
    din('traj_wT', (8, D))
    for nm2 in ('te1', 'te2', 'cg2', 'pproj', 'pp1', 'op1'):
        din(nm2, (NT, 128, D))
    din('cg1', (9, 128, D))
    din('enc_bias', (128, 12), f32)
    din('head_bias', (128, 9), f32)
    din('pp2T', (NT, 128, 7))
    din('op2T', (NT, 128, 1))
    din('pp2_b', (1, 7))
    din('op2_b', (1, 1))
    din('S_pad', (128, 128))
    din('S_nop', (128, 128))
    din('divmat_p', (3, DP))
    din('divmat_n', (3, D))
    din('sinembL', (NT, 128, L), f32)
    din('tfreq', (1, D))
    din('tphase', (128, 3), f32)
    dout('out', (L, 8))
    for tname in debug_taps:
        dout(f'tap_{tname}', _TAP_SHAPES[tname])

    def maybe_tap(name, ap):
        if name in debug_taps:
            nc.sync.dma_start(dram[f'tap_{name}'][:].bitcast(ap.dtype), ap)

    with tile.TileContext(nc) as tc, \
         nc.allow_low_precision(reason='f32r is full fp32 bits'), \
         tc.tile_pool(name='persist', bufs=1) as P, \
         tc.tile_pool(name='psA', bufs=4, space='PSUM') as PSA, \
         tc.tile_pool(name='psB', bufs=3, space='PSUM') as PSB, \
         tc.tile_pool(name='roptmp', bufs=2) as TMPR, \
         tc.tile_pool(name='pbuf', bufs=2) as PBP, \
         tc.tile_pool(name='lnt', bufs=2) as LNT:

        cnt = [0]

        def psum_mm(shape):
            cnt[0] += 1
            return PSA.tile(shape, f32, tag='mm', name=f'mm{cnt[0]}')

        def psum_sm(shape):
            cnt[0] += 1
            return PSB.tile(shape, f32, tag='sm', name=f'sm{cnt[0]}')

        def mmacc(ps, pairs):
            n = len(pairs)
            for i, (lhsT, rhs) in enumerate(pairs):
                nc.tensor.matmul(ps, lhsT, rhs, start=(i == 0),
                                 stop=(i == n - 1))

        def f32cast(ap):
            return ap.bitcast(f32) if ap.dtype == f32r else ap

        def mmacc32(ps, pairs):
            n = len(pairs)
            for i, (lhsT, rhs) in enumerate(pairs):
                nc.tensor.matmul(ps, f32cast(lhsT), f32cast(rhs),
                                 start=(i == 0), stop=(i == n - 1))

        def load_w(pool, name, tag):
            d = dram[name]
            sh = list(d.tensor.shape)
            if len(sh) == 3:
                t = pool.tile([128, sh[0], sh[2]], f32r, tag=tag)
                nc.sync.dma_start(t[:], d.rearrange('a p f -> p a f'))
            else:
                t = pool.tile(sh, f32 if d.tensor.dtype == f32 else f32r,
                              tag=tag)
                nc.sync.dma_start(t[:], d[:])
            return t

        # ---------------- constants ----------------
        S_pad = P.tile([128, 128], f32r, tag='S_pad')
        nc.sync.dma_start(S_pad[:], dram['S_pad'][:])
        S_pad_bf = P.tile([128, 128], bf16, tag='S_pad_bf')
        nc.vector.tensor_copy(S_pad_bf[:], S_pad[:])
        divp = P.tile([3, DP], f32r, tag='divp')
        nc.sync.dma_start(divp[:], dram['divmat_p'][:])
        sinembL = P.tile([128, NT, L], f32, tag='sinembL')
        nc.sync.dma_start(sinembL[:], dram['sinembL'].rearrange('a p f -> p a f'))
        tfreq = P.tile([1, D], f32r, tag='tfreq')
        nc.sync.dma_start(tfreq[:], dram['tfreq'][:])
        tphase = P.tile([128, 3], f32, tag='tphase')
        nc.sync.dma_start(tphase[:], dram['tphase'][:])
        ones_r = P.tile([1, 128], f32r, tag='ones_r')
        nc.vector.memset(ones_r[:].bitcast(f32), 1.0)
        ones_c = P.tile([128, 1], f32r, tag='ones_c')
        nc.vector.memset(ones_c[:].bitcast(f32), 1.0)
        halfpi = P.tile([128, 1], f32, tag='halfpi')
        nc.vector.memset(halfpi[:], float(np.pi / 2))
        epsc = P.tile([1, 1], f32, tag='epsc')
        nc.vector.memset(epsc[:], EPS)
        oneD_c = P.tile([128, 1], f32r, tag='oneD_c')
        nc.vector.memset(oneD_c[:].bitcast(f32), 1.0 / D)

        # ---------------- timestep/gripper embedding ----------------
        tstep = P.tile([1, 1], f32r, tag='tstep')
        nc.sync.dma_start(tstep[:], dram['tstep'][:])
        enc_b = P.tile([128, 12], f32, tag='enc_bias')
        nc.sync.dma_start(enc_b[:], dram['enc_bias'][:])
        tsin = P.tile([128, 3], f32r, tag='tsin')
        TWO_PI = float(2 * np.pi)
        PI = float(np.pi)
        C1 = 6.28125
        C2 = float(np.float32(2 * np.pi - C1))
        C3 = float(2 * np.pi - C1 - np.float32(2 * np.pi - C1))
        i32 = mybir.dt.int32
        for kt in range(NT):
            ps = psum_sm([128, 1])
            mmacc32(ps[:], [(tfreq[:, kt * 128:(kt + 1) * 128], tstep[:])])
            tv = TMPR.tile([128, 5], f32, tag='mcol', name='tv')
            ki = TMPR.tile([128, 1], i32, tag='mcoli', name='ki')
            xang, k4, kf, ycw, yw = (tv[:, 0:1], tv[:, 1:2], tv[:, 2:3],
                                     tv[:, 3:4], tv[:, 4:5])
            nc.vector.tensor_scalar_add(xang, ps[:], tphase[:, kt:kt + 1])
            nc.vector.tensor_scalar(k4, xang, 1.0 / TWO_PI, 0.5,
                                    ALU.mult, ALU.add)
            nc.vector.tensor_copy(ki[:], k4)
            nc.vector.tensor_copy(kf, ki[:])
            nc.vector.cody_waite_cascade(ycw, xang, kf, C1, C2, C3)
            nc.vector.add_range_wrap(yw, ycw, 0.0, PI, TWO_PI)
            nc.scalar.activation(tsin[:, kt:kt + 1], yw, AF.Sin)

        temb = P.tile([128, 3], f32r, tag='temb')
        st = P.tile([128, 3], f32r, tag='st')
        with tc.tile_pool(name='enc', bufs=1) as ENC:
            def vec_mlp(w1name, w2name, b_off, x_col, out_tile, nk=3):
                w1 = load_w(ENC, w1name, w1name)
                w2 = load_w(ENC, w2name, w2name)
                h = ENC.tile([128, 3], f32r, tag=f'h_{w1name}')
                for mt in range(NT):
                    ps = psum_sm([128, 1])
                    mmacc32(ps[:], [(w1[:, kk, mt * 128:(mt + 1) * 128],
                                     x_col[:, kk:kk + 1]) for kk in range(nk)])
                    nc.scalar.activation(h[:, mt:mt + 1], ps[:], AF.Relu,
                                         bias=enc_b[:, b_off + mt:b_off + mt + 1])
                for mt in range(NT):
                    ps = psum_sm([128, 1])
                    mmacc32(ps[:], [(w2[:, kk, mt * 128:(mt + 1) * 128],
                                     h[:, kk:kk + 1]) for kk in range(NT)])
                    nc.scalar.activation(out_tile[:, mt:mt + 1], ps[:],
                                         AF.Identity,
                                         bias=enc_b[:, b_off + 3 + mt:b_off + 4 + mt])

            tfe = ENC.tile([128, 3], f32r, tag='tfe')
            vec_mlp('te1', 'te2', 0, tsin, tfe)
            gflat = ENC.tile([128, 9], f32r, tag='gflat')
            nc.sync.dma_start(gflat[:],
                              dram['gflat'].rearrange('a p f -> p (a f)'))
            gfe = ENC.tile([128, 3], f32r, tag='gfe')
            vec_mlp('cg1', 'cg2', 6, gflat, gfe, nk=9)
            nc.vector.tensor_tensor(temb[:], tfe[:], gfe[:], ALU.add)
            sg = ENC.tile([128, 3], f32, tag='sg')
            nc.scalar.activation(sg[:], temb[:], AF.Sigmoid)
            nc.vector.tensor_tensor(st[:], temb[:], sg[:], ALU.mult)
        maybe_tap('temb', temb[:])

        ALL_BLOCKS = ['c0', 'c1', 's0', 's1', 's2', 's3', 'p0', 'p1']
        mods_all = P.tile([128, 12 * len(ALL_BLOCKS)], f32, tag='mods_all')
        with tc.tile_pool(name='adaw', bufs=2) as ADAW:
            for bi, nm in enumerate(ALL_BLOCKS):
                ada = load_w(ADAW, f'{nm}_ada', 'ada')
                adab = load_w(ADAW, f'{nm}_adab', 'adab')
                for mt in range(12):
                    ps = psum_sm([128, 1])
                    mmacc32(ps[:], [(ada[:, kk, mt * 128:(mt + 1) * 128],
                                     st[:, kk:kk + 1]) for kk in range(NT)])
                    nc.scalar.activation(
                        mods_all[:, bi * 12 + mt:bi * 12 + mt + 1], ps[:],
                        AF.Identity, bias=adab[:, mt:mt + 1])

        # ---------------- self/cross-q rope tables ----------------
        cos_s = P.tile([128, NTP, TS], f32r, tag='cos_s')
        sin_s = P.tile([128, NTP, TS], f32r, tag='sin_s')
        traj_xyz = P.tile([3, L], f32r, tag='traj_xyz')
        nc.sync.dma_start(traj_xyz[:], dram['traj_xyz'][:])
        for mt in range(NTP):
            ps = psum_mm([128, L])
            nc.tensor.matmul(ps[:], divp[:, mt * 128:(mt + 1) * 128],
                             traj_xyz[:], start=True, stop=True)
            for tab, ph in ((cos_s, float(np.pi / 2)), (sin_s, 0.0)):
                mt1 = TMPR.tile([128, 320], f32, tag='ropet1', name='mt1')
                nc.vector.add_range_wrap(mt1[:, 0:L], ps[:], ph, PI, TWO_PI)
                nc.scalar.activation(tab[:, mt, 0:L], mt1[:, 0:L], AF.Sin)
        nc.sync.dma_start(cos_s[:, :, L:TS],
                          dram['fps_cosT'].rearrange('a p f -> p a f'))
        nc.sync.dma_start(sin_s[:, :, L:TS],
                          dram['fps_sinT'].rearrange('a p f -> p a f'))

        # ---------------- trajectory encoder ----------------
        traj_aug = P.tile([8, L], f32r, tag='traj_aug')
        nc.sync.dma_start(traj_aug[:], dram['traj_aug'][:])
        traj_wT = P.tile([8, D], f32r, tag='traj_wT')
        nc.sync.dma_start(traj_wT[:], dram['traj_wT'][:])
        residA = P.tile([128, NT, TS], f32r, tag='residA')
        residB = P.tile([128, NT, TS], f32r, tag='residB')
        for mt in range(NT):
            ps = psum_mm([128, L])
            nc.tensor.matmul(ps[:], traj_wT[:, mt * 128:(mt + 1) * 128],
                             traj_aug[:], start=True, stop=True)
            nc.vector.tensor_tensor(residA[:, mt, 0:L], ps[:],
                                    sinembL[:, mt, :], ALU.add)
        maybe_tap('trajenc', residA[:, :, 0:L])

        # ---------------- shared block pieces ----------------
        def adaln_mods(nm, TMP):
            bi = ALL_BLOCKS.index(nm)
            mods = mods_all[:, bi * 12:(bi + 1) * 12]
            s1 = TMP.tile([128, 3], f32, tag='s1')
            nc.vector.tensor_scalar_add(s1[:], mods[:, 0:3], 1.0)
            s1f = TMP.tile([128, 3], f32, tag='s1f')
            nc.vector.tensor_scalar_add(s1f[:], mods[:, 6:9], 1.0)
            return mods, s1, s1f

        def layer_norm(x_in, x_out, bias, boff, T, TMP):
            ncs = _nc_chunks(T)
            stats = TMP.tile([1, 2 * T], f32, tag='lnstat', name='lnstat')
            work = TMP.tile([1, T], f32, tag='lnwork', name='lnwork')
            rr = TMP.tile([1, 2 * T], f32r, tag='lnrr', name='lnrr')
            m1, e1 = stats[:, 0:T], stats[:, T:2 * T]
            for (o, c) in ncs:
                ps = psum_sm([1, c])
                mmacc(ps[:], [(oneD_c[:], x_in[:, kt, o:o + c])
                              for kt in range(NT)])
                nc.vector.tensor_copy(m1[:, o:o + c], ps[:])
                ps2 = psum_sm([1, c])
                for kt in range(NT):
                    sqc = LNT.tile([128, 512], f32r, tag='lnsq', name='sqc')
                    nc.scalar.activation(sqc[:, 0:c], x_in[:, kt, o:o + c],
                                         AF.Square)
                    nc.tensor.matmul(ps2[:], oneD_c[:], sqc[:, 0:c],
                                     start=(kt == 0), stop=(kt == NT - 1))
                nc.vector.tensor_copy(e1[:, o:o + c], ps2[:])
            nc.vector.scalar_tensor_tensor(work[:], m1, 1.0, m1,
                                           ALU.mult, ALU.mult)
            nc.vector.tensor_tensor(e1, e1, work[:], ALU.subtract)
            nc.scalar.activation(work[:], e1, AF.Sqrt, bias=epsc[:])
            rstd, mr = rr[:, 0:T], rr[:, T:2 * T]
            nc.vector.reciprocal(rstd, work[:])
            nc.vector.tensor_tensor(mr, m1, rstd, ALU.mult)
            for (o, c) in ncs:
                psr = psum_mm([128, c])
                nc.tensor.matmul(psr[:], ones_r[:], rstd[:, o:o + c],
                                 start=True, stop=True)
                psm = psum_mm([128, c])
                nc.tensor.matmul(psm[:], ones_r[:], mr[:, o:o + c],
                                 start=True, stop=True)
                for kt in range(NT):
                    t1 = LNT.tile([128, 512], f32, tag='lnt1', name='lnt1')
                    nc.vector.tensor_tensor(t1[:, 0:c], x_in[:, kt, o:o + c],
                                            psr[:], ALU.mult)
                    nc.vector.tensor_tensor(t1[:, 0:c], t1[:, 0:c], psm[:],
                                            ALU.subtract)
                    nc.scalar.activation(
                        x_out[:, kt, o:o + c], t1[:, 0:c], AF.Identity,
                        bias=bias[:, boff + 3 + kt:boff + 4 + kt],
                        scale=bias[:, boff + kt:boff + 1 + kt])

        def attn_block(nm, x_resid, x_out, T, cross, WPb, TMP,
                       ctx_res=None):
            Tkv = N if cross else T
            bias = load_w(WPb, f'{nm}_bias', 'bias')
            wq = load_w(WPb, f'{nm}_wq', 'wq')
            wk = load_w(WPb, f'{nm}_wk', 'wk')
            wv = load_w(WPb, f'{nm}_wv', 'wv')
            wo = load_w(WPb, f'{nm}_wo', 'wo')
            w1 = load_w(WPb, f'{nm}_w1', 'w1')
            w2 = load_w(WPb, f'{nm}_w2', 'w2')
            bv = load_w(WPb, f'{nm}_bv', 'bv')
            mods, s1, s1f = adaln_mods(nm, TMP)

            ncq = _nc_chunks(T)
            nckv = _nc_chunks(Tkv)
            kvc = _kv_chunks(Tkv)
            nkt = NTP
            pdt = bf16 if cross else f32r
            Smat = S_pad_bf if cross else S_pad

            # adaln on q input
            aq = TMP.tile([128, NT, T], f32r, tag='gp1')
            for kt in range(NT):
                nc.scalar.activation(aq[:, kt, :], x_resid[:, kt, :],
                                     AF.Identity, bias=mods[:, 3 + kt:4 + kt],
                                     scale=s1[:, kt:kt + 1])

            # q projection (padded) + in-place rope
            qt = TMP.tile([128, NTP, T], pdt, tag='qt')
            for mt in range(NTP):
                for (o, c) in ncq:
                    ps = psum_mm([128, c])
                    mmacc(ps[:], [(wq[:, kt, mt * 128:(mt + 1) * 128],
                                   aq[:, kt, o:o + c]) for kt in range(NT)])
                    nc.scalar.activation(qt[:, mt, o:o + c], ps[:],
                                         AF.Identity, bias=bias[:, mt:mt + 1])
                for (o, c) in ncq:
                    pss = psum_mm([128, c])
                    nc.tensor.matmul(pss[:], Smat[:], qt[:, mt, o:o + c],
                                     start=True, stop=True)
                    t1 = TMPR.tile([128, 320], f32, tag='ropet1')
                    nc.vector.tensor_tensor(t1[:, 0:c], qt[:, mt, o:o + c],
                                            cos_s[:, mt, o:o + c], ALU.mult)
                    t2 = TMPR.tile([128, 320], f32, tag='ropet2')
                    nc.vector.tensor_tensor(t2[:, 0:c], pss[:],
                                            sin_s[:, mt, o:o + c], ALU.mult)
                    nc.vector.tensor_tensor(qt[:, mt, o:o + c], t1[:, 0:c],
                                            t2[:, 0:c], ALU.add)
            maybe_tap(f'{nm}_qrot', qt[:])

            # k projection + in-place rope
            if cross:
                ksrc, ctxT, ctx_xyz, krot, va = (
                    ctx_res['ctxT'], ctx_res['ctxT'], ctx_res['ctx_xyz'],
                    ctx_res['ctx_k'], ctx_res['ctx_va'])
            else:
                ksrc = x_resid
                krot = TMP.tile([128, NTP, T], f32r, tag='krot')
                va = TMP.tile([128, len(kvc), H, 65], f32r, tag='va')
            for mt in range(nkt):
                for (o, c) in nckv:
                    ps = psum_mm([128, c])
                    mmacc(ps[:], [(wk[:, kt, mt * 128:(mt + 1) * 128],
                                   ksrc[:, kt, o:o + c]) for kt in range(NT)])
                    nc.scalar.activation(krot[:, mt, o:o + c], ps[:],
                                         AF.Identity, bias=bias[:, 4 + mt:5 + mt])
                for (o, c) in nckv:
                    if cross:
                        psa = psum_mm([128, c])
                        nc.tensor.matmul(psa[:],
                                         divp[:, mt * 128:(mt + 1) * 128],
                                         ctx_xyz[:, o:o + c],
                                         start=True, stop=True)
                        cosk = ctx_res['CROPE'].tile([128, 320], f32,
                                                     tag='cosk', name='cosk')
                        nc.vector.add_range_wrap(cosk[:, 0:c], psa[:],
                                                 float(np.pi / 2), PI, TWO_PI)
                        nc.scalar.activation(cosk[:, 0:c], cosk[:, 0:c],
                                             AF.Sin)
                        sink = ctx_res['CROPE'].tile([128, 320], f32,
                                                     tag='sink', name='sink')
                        nc.vector.add_range_wrap(sink[:, 0:c], psa[:],
                                                 0.0, PI, TWO_PI)
                        nc.scalar.activation(sink[:, 0:c], sink[:, 0:c],
                                             AF.Sin)
                        cos_ap, sin_ap = cosk[:, 0:c], sink[:, 0:c]
                    else:
                        cos_ap = cos_s[:, mt, o:o + c]
                        sin_ap = sin_s[:, mt, o:o + c]
                    pss = psum_mm([128, c])
                    nc.tensor.matmul(pss[:], Smat[:], krot[:, mt, o:o + c],
                                     start=True, stop=True)
                    t1 = TMPR.tile([128, 320], f32, tag='ropet1')
                    nc.vector.tensor_tensor(t1[:, 0:c], krot[:, mt, o:o + c],
                                            cos_ap, ALU.mult)
                    t2 = TMPR.tile([128, 320], f32, tag='ropet2')
                    nc.vector.tensor_tensor(t2[:, 0:c], pss[:], sin_ap,
                                            ALU.mult)
                    nc.vector.tensor_tensor(krot[:, mt, o:o + c], t1[:, 0:c],
                                            t2[:, 0:c], ALU.add)
            maybe_tap(f'{nm}_krot', krot[:])

            # v projection (token-major) + ones column at 64
            nc.vector.memset(va[:, :, :, HD:65].bitcast(f32)
                 if not cross else va[:, :, :, HD:65], 1.0)
            for ci, (o, c) in enumerate(kvc):
                ps = psum_mm([128, D])
                pairs = [(ksrc[:, kt, o:o + c], wv[:, kt, :])
                         for kt in range(NT)]
                pairs.append((ones_r[:, 0:c], bv[:]))
                mmacc(ps[0:c, :], pairs)
                nc.scalar.activation(
                    va[0:c, ci, :, 0:HD],
                    ps[0:c, :].rearrange('p (h d) -> p h d', h=H), AF.Copy)

            # attention (attout in padded head layout; pad rows are garbage
            # but multiply against zero rows of the padded Wo)
            attout = TMP.tile([128, NTP, T], f32r, tag='gp1', name='attout')
            nc.vector.memset(attout[32:64, :, :].bitcast(f32), 0.0)
            nc.vector.memset(attout[96:128, :, :].bitcast(f32), 0.0)
            for h in range(H):
                mt_q, off_q = h // 2, (h % 2) * 64
                for (oq, cq) in ncq:
                    Pb = PBP.tile([128, len(kvc), cq], pdt, tag='P')
                    for ci, (o, c) in enumerate(kvc):
                        ps = psum_mm([128, cq])
                        pairs = [(krot[off_q:off_q + HD, mt_q, o:o + c],
                                  qt[off_q:off_q + HD, mt_q, oq:oq + cq])]
                        mmacc(ps[0:c, :], pairs)
                        nc.scalar.activation(Pb[0:c, ci, 0:cq], ps[0:c, :],
                                             AF.Exp)
                    pav = psum_sm([65, cq])
                    mmacc(pav[:], [(va[0:c, ci, h, :], Pb[0:c, ci, 0:cq])
                                   for ci, (o, c) in enumerate(kvc)])
                    inv = TMPR.tile([1, 288], f32r, tag='inv')
                    nc.vector.reciprocal(inv[:, 0:cq], pav[64:65, :])
                    pb = psum_sm([HD, cq])
                    nc.tensor.matmul(pb[:], ones_r[:, 0:HD], inv[:, 0:cq],
                                     start=True, stop=True)
                    nc.scalar.activation(
                        attout[off_q:off_q + HD, mt_q, oq:oq + cq],
                        pav[0:HD, :], AF.Copy)
                    nc.vector.tensor_tensor(
                        attout[off_q:off_q + HD, mt_q, oq:oq + cq],
                        attout[off_q:off_q + HD, mt_q, oq:oq + cq],
                        pb[:], ALU.mult)
            maybe_tap(f'{nm}_attout', attout[:])

            # out projection + bias + residual
            x1 = TMP.tile([128, NT, T], f32r, tag='gp2')
            for mt in range(NT):
                for (o, c) in ncq:
                    ps = psum_mm([128, c])
                    mmacc(ps[:], [(wo[:, kt, mt * 128:(mt + 1) * 128],
                                   attout[:, kt, o:o + c])
                                  for kt in range(NTP)])
                    nc.vector.scalar_tensor_tensor(
                        x1[:, mt, o:o + c], ps[:], bias[:, 8 + mt:9 + mt],
                        x_resid[:, mt, o:o + c], ALU.add, ALU.add)

            xl = TMP.tile([128, NT, T], f32r, tag='xl')
            layer_norm(x1, xl, bias, 29, T, TMP)
            maybe_tap(f'{nm}_xl', xl[:])

            x2 = TMP.tile([128, NT, T], f32r, tag='gp1')
            for kt in range(NT):
                nc.scalar.activation(x2[:, kt, :], xl[:, kt, :], AF.Identity,
                                     bias=mods[:, 9 + kt:10 + kt],
                                     scale=s1f[:, kt:kt + 1])
            hbuf = TMP.tile([128, NT, T], f32r, tag='gp2')
            for mt in range(NT):
                for (o, c) in ncq:
                    ps = psum_mm([128, c])
                    mmacc(ps[:], [(w1[:, kt, mt * 128:(mt + 1) * 128],
                                   x2[:, kt, o:o + c]) for kt in range(NT)])
                    nc.scalar.activation(hbuf[:, mt, o:o + c], ps[:], AF.Relu,
                                         bias=bias[:, 11 + mt:12 + mt])
            x3 = TMP.tile([128, NT, T], f32r, tag='gp1')
            for mt in range(NT):
                for (o, c) in ncq:
                    ps = psum_mm([128, c])
                    mmacc(ps[:], [(w2[:, kt, mt * 128:(mt + 1) * 128],
                                   hbuf[:, kt, o:o + c]) for kt in range(NT)])
                    nc.vector.scalar_tensor_tensor(
                        x3[:, mt, o:o + c], ps[:], bias[:, 14 + mt:15 + mt],
                        xl[:, mt, o:o + c], ALU.add, ALU.add)
            layer_norm(x3, x_out, bias, 35, T, TMP)
            maybe_tap(f'{nm}_out', x_out[:, :, 0:T])

        # ---------------- cross phase ----------------
        with tc.tile_pool(name='wcross', bufs=1) as WPC, \
             tc.tile_pool(name='tmpc', bufs=1) as TMPC, \
             tc.tile_pool(name='crope', bufs=2) as CROPE, \
             tc.tile_pool(name='ctxp', bufs=1) as CP:
            ctx_res = {
                'CROPE': CROPE,
                'ctxT': CP.tile([128, NT, N], f32r, tag='ctxT',
                                name='ctxT'),
                'ctx_xyz': CP.tile([3, N], f32r, tag='ctx_xyz',
                                   name='ctx_xyz'),
                'ctx_k': CP.tile([128, NTP, N], bf16, tag='ctx_k',
                                 name='ctx_k'),
                'ctx_va': CP.tile([128, N // 128, H, 65], bf16,
                                  tag='ctx_va', name='ctx_va'),
            }
            nc.sync.dma_start(ctx_res['ctxT'][:],
                              dram['ctxT'].rearrange('a p f -> p a f'))
            nc.sync.dma_start(ctx_res['ctx_xyz'][:], dram['ctx_xyz'][:])
            attn_block('c0', residA[:, :, 0:L], residB[:, :, 0:L], L,
                       True, WPC, TMPC, ctx_res)
            attn_block('c1', residB[:, :, 0:L], residA[:, :, 0:L], L,
                       True, WPC, TMPC, ctx_res)

        # feats assembly: cross output already in residA cols 0:L
        nc.sync.dma_start(residA[:, :, L:TS],
                          dram['fpsT'].rearrange('a p f -> p a f'))
        maybe_tap('feats', residA[:])

        # ---------------- self phase + heads ----------------
        with tc.tile_pool(name='wself', bufs=2) as WPS, \
             tc.tile_pool(name='tmps', bufs=1) as TMPS:
            cur, nxt = residA, residB
            for nm in ('s0', 's1', 's2', 's3', 'p0', 'p1'):
                attn_block(nm, cur, nxt, TS, False, WPS, TMPS)
                cur, nxt = nxt, cur

            head_b = P.tile([128, 9], f32, tag='head_bias')
            nc.sync.dma_start(head_b[:], dram['head_bias'][:])
            posf = cur

            hw_tags = {'pproj': 'wv', 'pp1': 'wo', 'op1': 'w1'}

            def head_proj(wname, boff, src, func, Tsrc):
                wt = load_w(WPS, wname, hw_tags[wname])
                out = TMPS.tile([128, NT, L], f32r, tag=f'h_{wname}')
                for mt in range(NT):
                    ps = psum_mm([128, L])
                    mmacc(ps[:], [(wt[:, kt, mt * 128:(mt + 1) * 128],
                                   src[:, kt, 0:L]) for kt in range(NT)])
                    nc.scalar.activation(out[:, mt, :], ps[:], func,
                                         bias=head_b[:, boff + mt:boff + mt + 1])
                return out

            pf = head_proj('pproj', 0, posf, AF.Identity, TS)
            maybe_tap('posf', pf[:])
            h1 = head_proj('pp1', 3, pf, AF.Relu, L)
            h2 = head_proj('op1', 6, pf, AF.Relu, L)
            pp2T = load_w(WPS, 'pp2T', 'w2')
            op2T = load_w(WPS, 'op2T', 'wk')
            pp2b = load_w(WPS, 'pp2_b', 'bv')
            op2b = load_w(WPS, 'op2_b', 'bv')
            outsb = TMPS.tile([L, 8], f32, tag='outsb')
            ps = psum_sm([L, 7])
            mmacc32(ps[:], [(h1[:, kt, :], pp2T[:, kt, :])
                            for kt in range(NT)]
                    + [(ones_r[:, 0:L], pp2b[:])])
            nc.scalar.copy(outsb[:, 0:7], ps[:])
            ps2 = psum_sm([L, 1])
            mmacc32(ps2[:], [(h2[:, kt, :], op2T[:, kt, :])
                             for kt in range(NT)]
                     + [(ones_r[:, 0:L], op2b[:])])
            nc.scalar.copy(outsb[:, 7:8], ps2[:])
            nc.sync.dma_start(dram['out'][:], outsb[:])

    nc.compile()
    bd.dram = dram
    return bd


# ------------------------------------------------------------------- entry
_CACHE = {}


def kernel(**inputs):
    key = 'bd' + ','.join(sorted(DEBUG_TAPS))
    if key not in _CACHE:
        _CACHE[key] = build(tuple(DEBUG_TAPS))
    bd = _CACHE[key]
    w = prep_weights(inputs['params'])
    in_maps = []
    for b in range(N_CORES):
        m = dict(w)
        m.update(prep_core_inputs(b, inputs))
        in_maps.append(m)
    res = run_bass_kernel_spmd(bd.nc, in_maps, core_ids=list(range(N_CORES)))
    _CACHE['last_results'] = res
    out = np.stack([res.results[b]['out'] for b in range(N_CORES)], axis=0)
    return out.astype(np.float32)


# revision 27
# speedup vs baseline: 1.0537x; 1.0537x over previous
"""Trainium2 Bass kernel for nn_DiffuserJointer (dense diffusion transformer).

Strategy: data-parallel over batch B=8 across 8 NeuronCores (one batch
element per core; no collectives). On-core layout is feature-major
(features on partitions, tokens on the free dim) so every matmul contracts
over partitions with no transposes. All matmuls run in float32r (fp32 bits,
~tf32 precision, bf16-rate on the PE). Attention uses a transposed-scores
formulation (kv on partitions) so softmax normalization folds into an
augmented ones-column of V; RoPE's pair-shuffle is a constant 128x128
matmul; per-token normalizers broadcast via rank-1 matmuls.
"""
import sys
sys.path.insert(0, '/opt/trn_rl_repo')

import numpy as np

import concourse.bass as bass
import concourse.bacc as bacc
import concourse.mybir as mybir
import concourse.tile as tile
from concourse.bass_utils import run_bass_kernel_spmd

f32 = mybir.dt.float32
f32r = mybir.dt.float32r
bf16 = mybir.dt.bfloat16
AF = mybir.ActivationFunctionType
ALU = mybir.AluOpType

# ---- problem dims (hardcoded) ----
D, H, B, L, N, K, NHIST = 384, 8, 8, 64, 2560, 512, 3
HD = D // H            # 48
DP = 512               # padded q/k feature dim (64 per head)
TS = L + K             # 576 self-attn tokens
NT = 3                 # feature tiles of 128
NTP = 4                # padded feature tiles
N_CORES = 8
EPS = 1e-5

# optional debug taps: list of names filled by the test harness before build
DEBUG_TAPS = []
_TAP_SHAPES = {}


# ---------------------------------------------------------------- host prep
def _sin_emb_table(x, dim):
    half = dim // 2
    freqs = np.exp(-np.log(10000.0) / (half - 1) * np.arange(half, dtype=np.float64))
    ang = np.asarray(x, np.float64)[:, None] * freqs[None, :]
    return np.concatenate([np.sin(ang), np.cos(ang)], axis=-1).astype(np.float32)


def _rope_div():
    d = D // 3  # 128
    return np.exp(np.arange(0, d, 2, dtype=np.float64)
                  * (-np.log(10000.0) / d)).astype(np.float32)


def _pad_rows(w):
    """(384, ...) head rows -> (512, ...): head h rows h*48..+47 -> h*64..+47."""
    out = np.zeros((DP,) + w.shape[1:], dtype=np.float32)
    for h in range(H):
        out[h * 64:h * 64 + HD] = w[h * HD:(h + 1) * HD]
    return out


def _tile_lhsT(wT):
    din = wT.shape[0]
    assert din % 128 == 0
    return np.ascontiguousarray(wT.reshape(din // 128, 128, *wT.shape[1:]))


def _cols(v):
    return np.ascontiguousarray(np.asarray(v, np.float32).reshape(NT, 128).T)


def _cols_p(v):
    return np.ascontiguousarray(np.asarray(v, np.float32).reshape(NTP, 128).T)


def _shuffle_mat(padded):
    S = np.zeros((128, 128), dtype=np.float32)
    if padded:
        for b0 in (0, 64):
            for i in range(0, HD, 2):
                S[b0 + i, b0 + i + 1] = -1.0
                S[b0 + i + 1, b0 + i] = 1.0
    else:
        for i in range(0, 128, 2):
            S[i, i + 1] = -1.0
            S[i + 1, i] = 1.0
    return np.ascontiguousarray(S.T)


def _divmat(padded):
    div = _rope_div()
    n = DP if padded else D
    M = np.zeros((3, n), dtype=np.float32)
    for r in range(n):
        if padded:
            h, j = r // 64, r % 64
            if j >= HD:
                continue
            d = h * HD + j
        else:
            d = r
        M[d // 128, r] = div[(d % 128) // 2]
    return M


def _pad_feat_T(x):
    """(T, 384) -> (512, T) padded feature-major."""
    xT = np.zeros((DP, x.shape[0]), dtype=np.float32)
    for h in range(H):
        xT[h * 64:h * 64 + HD] = x[:, h * HD:(h + 1) * HD].T
    return np.ascontiguousarray(xT)


def prep_weights(params):
    p = {k: (np.asarray(v, np.float32) if not isinstance(v, (list, dict)) else v)
         for k, v in params.items()}
    w = {}

    def blk(name, bp, cross):
        bp = {k: np.asarray(v, np.float32) for k, v in bp.items()}
        wq, wk, wv = bp['in_w'][:D], bp['in_w'][D:2 * D], bp['in_w'][2 * D:]
        bq, bk, bv = bp['in_b'][:D], bp['in_b'][D:2 * D], bp['in_b'][2 * D:]
        sc = HD ** -0.5
        wq_p, bq_p = _pad_rows(wq * sc), _pad_rows(bq * sc)
        w[f'{name}_wq'] = _tile_lhsT(wq_p.T.copy())           # (3,128,512)
        w[f'{name}_wk'] = _tile_lhsT(_pad_rows(wk).T.copy())  # (3,128,512)
        bk_cols = _cols_p(_pad_rows(bk))
        w[f'{name}_wv'] = _tile_lhsT(wv.T.copy())             # (3,128,384)
        w[f'{name}_wo'] = _tile_lhsT(_pad_rows(bp['out_w'].T.copy()))
        w[f'{name}_w1'] = _tile_lhsT(bp['w1'].T.copy())
        w[f'{name}_w2'] = _tile_lhsT(bp['w2'].T.copy())
        ada_cat = np.concatenate([bp['ada_w'], bp['f_ada_w']], axis=0)
        w[f'{name}_ada'] = _tile_lhsT(ada_cat.T.copy())       # (3,128,1536)
        cols = [
            _cols_p(bq_p),                               # 0:4
            bk_cols,                                     # 4:8
            _cols(bp['out_b']),                          # 8:11
            _cols(bp['b1']),                             # 11:14
            _cols(bp['b2']),                             # 14:17
            np.ascontiguousarray(np.concatenate(
                [bp['ada_b'], bp['f_ada_b']]).reshape(12, 128).T),  # 17:29
            _cols(bp['ln1_g']), _cols(bp['ln1_b']),      # 29:32, 32:35
            _cols(bp['ln2_g']), _cols(bp['ln2_b']),      # 35:38, 38:41
        ]
        w[f'{name}_bias'] = np.ascontiguousarray(np.concatenate(cols, axis=1))
        w[f'{name}_bv'] = np.ascontiguousarray(bv.reshape(1, D))
        w[f'{name}_adab'] = np.ascontiguousarray(np.concatenate(
            [bp['ada_b'], bp['f_ada_b']]).reshape(12, 128).T)

    for i, bp in enumerate(p['cross']):
        blk(f'c{i}', bp, cross=True)
    for i, bp in enumerate(p['selfa']):
        blk(f's{i}', bp, cross=False)
    for i, bp in enumerate(p['posa']):
        blk(f'p{i}', bp, cross=False)

    traj_aug = np.concatenate([p['traj_w'], p['traj_b'][:, None]], axis=1)
    w['traj_wT'] = np.ascontiguousarray(traj_aug.T)     # (8, 384)
    w['te1'] = _tile_lhsT(p['te1_w'].T.copy())
    w['te2'] = _tile_lhsT(p['te2_w'].T.copy())
    w['cg1'] = _tile_lhsT(p['cg1_w'].T.copy())          # (9,128,384)
    w['cg2'] = _tile_lhsT(p['cg2_w'].T.copy())
    w['enc_bias'] = np.ascontiguousarray(np.concatenate(
        [_cols(p['te1_b']), _cols(p['te2_b']), _cols(p['cg1_b']),
         _cols(p['cg2_b'])], axis=1))                   # (128, 12)
    w['pproj'] = _tile_lhsT(p['pproj_w'].T.copy())
    w['pp1'] = _tile_lhsT(p['pp1_w'].T.copy())
    w['op1'] = _tile_lhsT(p['op1_w'].T.copy())
    w['head_bias'] = np.ascontiguousarray(np.concatenate(
        [_cols(p['pproj_b']), _cols(p['pp1_b']), _cols(p['op1_b'])], axis=1))
    w['pp2T'] = _tile_lhsT(p['pp2_w'].T.copy())         # (3,128,7)
    w['op2T'] = _tile_lhsT(p['op2_w'].T.copy())         # (3,128,1)
    w['pp2_b'] = np.ascontiguousarray(p['pp2_b'].reshape(1, 7))
    w['op2_b'] = np.ascontiguousarray(p['op2_b'].reshape(1, 1))

    w['S_pad'] = _shuffle_mat(True)
    w['S_nop'] = _shuffle_mat(False)
    w['divmat_p'] = _divmat(True)                        # (3,512)
    w['divmat_n'] = _divmat(False)                       # (3,384)
    w['sinembL'] = _tile_lhsT(np.ascontiguousarray(
        _sin_emb_table(np.arange(L), D).T))              # (3,128,64)
    half = D // 2
    tf = np.exp(-np.log(10000.0) / (half - 1)
                * np.arange(half)).astype(np.float32)
    w['tfreq'] = np.ascontiguousarray(np.concatenate([tf, tf]).reshape(1, D))
    phase = np.zeros(D, np.float32)
    phase[half:] = np.pi / 2
    w['tphase'] = _cols(phase)                           # (128,3)
    return w


def prep_core_inputs(b, inputs):
    m = {}
    traj = np.asarray(inputs['trajectory'], np.float32)[b]          # (64,7)
    m['traj_aug'] = np.ascontiguousarray(
        np.concatenate([traj.T, np.ones((1, L), np.float32)], axis=0))
    m['traj_xyz'] = np.ascontiguousarray(traj[:, :3].T)             # (3,64)
    m['tstep'] = np.array([[float(np.asarray(inputs['timestep'])[b])]],
                          np.float32)
    m['ctxT'] = np.ascontiguousarray(
        np.asarray(inputs['context_feats'], np.float32)[b].T
        .reshape(NT, 128, N))
    m['ctx_xyz'] = np.ascontiguousarray(
        np.asarray(inputs['context'], np.float32)[b].T)             # (3,2560)
    m['gflat'] = np.ascontiguousarray(
        np.asarray(inputs['adaln_gripper_feats'], np.float32)[b]
        .reshape(-1).reshape(9, 128, 1))
    m['fpsT'] = np.ascontiguousarray(
        np.asarray(inputs['fps_feats'], np.float32)[:, b, :].T
        .reshape(NT, 128, K))
    fp = np.asarray(inputs['fps_pos'], np.float32)[b]               # (512,384,2)
    m['fps_cosT'] = np.ascontiguousarray(
        _pad_feat_T(fp[:, :, 0]).reshape(NTP, 128, K))
    m['fps_sinT'] = np.ascontiguousarray(
        _pad_feat_T(fp[:, :, 1]).reshape(NTP, 128, K))
    return m


# ------------------------------------------------------------- bass builder
def _nc_chunks(T):
    if T <= 512:
        return [(0, T)]
    if T == TS:
        return [(0, 288), (288, 288)]
    return [(i, 320) for i in range(0, T, 320)]


def _kv_chunks(T):
    out, i = [], 0
    while i < T:
        c = min(128, T - i)
        out.append((i, c))
        i += c
    return out


def _head_rows(h, padded):
    if padded:
        return [(h // 2, (h % 2) * 64, HD)]
    lo, hi = h * HD, (h + 1) * HD
    out = []
    for t in range(lo // 128, (hi - 1) // 128 + 1):
        s = max(lo, t * 128) - t * 128
        e = min(hi, (t + 1) * 128) - t * 128
        out.append((t, s, e - s))
    return out


class _BD:
    pass


def build(debug_taps=()):
    bd = _BD()
    nc = bacc.Bacc('TRN2', target_bir_lowering=False, debug=False,
                   num_devices=N_CORES)
    bd.nc = nc
    dram = {}

    def din(name, shape, dtype=f32r):
        dram[name] = nc.dram_tensor(name, list(shape), dtype,
                                    kind='ExternalInput').ap()

    def dout(name, shape, dtype=f32):
        dram[name] = nc.dram_tensor(name, list(shape), dtype,
                                    kind='ExternalOutput').ap()

    din('traj_aug', (8, L))
    din('traj_xyz', (3, L))
    din('tstep', (1, 1))
    din('ctxT', (NT, 128, N))
    din('ctx_xyz', (3, N))
    din('gflat', (9, 128, 1))
    din('fpsT', (NT, 128, K))
    din('fps_cosT', (NTP, 128, K))
    din('fps_sinT', (NTP, 128, K))
    blocks = ['c0', 'c1', 's0', 's1', 's2', 's3', 'p0', 'p1']
    for nm in blocks:
        cross = nm.startswith('c')
        din(f'{nm}_wq', (NT, 128, DP))
        din(f'{nm}_wk', (NT, 128, DP))
        din(f'{nm}_wv', (NT, 128, D))
        din(f'{nm}_wo', (NTP, 128, D))
        din(f'{nm}_w1', (NT, 128, D))
        din(f'{nm}_w2', (NT, 128, D))
        din(f'{nm}_ada', (NT, 128, 4 * D))
        din(f'{nm}_bias', (128, 41), f32)
        din(f'{nm}_adab', (128, 12), f32)
        din(f'{nm}_bv', (1, D))
    din('traj_wT', (8, D))
    for nm2 in ('te1', 'te2', 'cg2', 'pproj', 'pp1', 'op1'):
        din(nm2, (NT, 128, D))
    din('cg1', (9, 128, D))
    din('enc_bias', (128, 12), f32)
    din('head_bias', (128, 9), f32)
    din('pp2T', (NT, 128, 7))
    din('op2T', (NT, 128, 1))
    din('pp2_b', (1, 7))
    din('op2_b', (1, 1))
    din('S_pad', (128, 128))
    din('S_nop', (128, 128))
    din('divmat_p', (3, DP))
    din('divmat_n', (3, D))
    din('sinembL', (NT, 128, L), f32)
    din('tfreq', (1, D))
    din('tphase', (128, 3), f32)
    dout('out', (L, 8))
    for tname in debug_taps:
        dout(f'tap_{tname}', _TAP_SHAPES[tname])

    def maybe_tap(name, ap):
        if name in debug_taps:
            nc.sync.dma_start(dram[f'tap_{name}'][:].bitcast(ap.dtype), ap)

    with tile.TileContext(nc) as tc, \
         nc.allow_low_precision(reason='f32r is full fp32 bits'), \
         tc.tile_pool(name='persist', bufs=1) as P, \
         tc.tile_pool(name='psA', bufs=4, space='PSUM') as PSA, \
         tc.tile_pool(name='psB', bufs=3, space='PSUM') as PSB, \
         tc.tile_pool(name='roptmp', bufs=2) as TMPR, \
         tc.tile_pool(name='pbuf', bufs=2) as PBP, \
         tc.tile_pool(name='lnt', bufs=2) as LNT:

        cnt = [0]

        def psum_mm(shape):
            cnt[0] += 1
            return PSA.tile(shape, f32, tag='mm', name=f'mm{cnt[0]}')

        def psum_sm(shape):
            cnt[0] += 1
            return PSB.tile(shape, f32, tag='sm', name=f'sm{cnt[0]}')

        def mmacc(ps, pairs):
            n = len(pairs)
            for i, (lhsT, rhs) in enumerate(pairs):
                nc.tensor.matmul(ps, lhsT, rhs, start=(i == 0),
                                 stop=(i == n - 1))

        def f32cast(ap):
            return ap.bitcast(f32) if ap.dtype == f32r else ap

        def mmacc32(ps, pairs):
            n = len(pairs)
            for i, (lhsT, rhs) in enumerate(pairs):
                nc.tensor.matmul(ps, f32cast(lhsT), f32cast(rhs),
                                 start=(i == 0), stop=(i == n - 1))

        def load_w(pool, name, tag):
            d = dram[name]
            sh = list(d.tensor.shape)
            if len(sh) == 3:
                t = pool.tile([128, sh[0], sh[2]], f32r, tag=tag)
                nc.sync.dma_start(t[:], d.rearrange('a p f -> p a f'))
            else:
                t = pool.tile(sh, f32 if d.tensor.dtype == f32 else f32r,
                              tag=tag)
                nc.sync.dma_start(t[:], d[:])
            return t

        # ---------------- constants ----------------
        S_pad = P.tile([128, 128], f32r, tag='S_pad')
        nc.sync.dma_start(S_pad[:], dram['S_pad'][:])
        S_pad_bf = P.tile([128, 128], bf16, tag='S_pad_bf')
        nc.vector.tensor_copy(S_pad_bf[:], S_pad[:])
        divp = P.tile([3, DP], f32r, tag='divp')
        nc.sync.dma_start(divp[:], dram['divmat_p'][:])
        sinembL = P.tile([128, NT, L], f32, tag='sinembL')
        nc.sync.dma_start(sinembL[:], dram['sinembL'].rearrange('a p f -> p a f'))
        tfreq = P.tile([1, D], f32r, tag='tfreq')
        nc.sync.dma_start(tfreq[:], dram['tfreq'][:])
        tphase = P.tile([128, 3], f32, tag='tphase')
        nc.sync.dma_start(tphase[:], dram['tphase'][:])
        ones_r = P.tile([1, 128], f32r, tag='ones_r')
        nc.vector.memset(ones_r[:].bitcast(f32), 1.0)
        ones_c = P.tile([128, 1], f32r, tag='ones_c')
        nc.vector.memset(ones_c[:].bitcast(f32), 1.0)
        halfpi = P.tile([128, 1], f32, tag='halfpi')
        nc.vector.memset(halfpi[:], float(np.pi / 2))
        epsc = P.tile([1, 1], f32, tag='epsc')
        nc.vector.memset(epsc[:], EPS)
        oneD_c = P.tile([128, 1], f32r, tag='oneD_c')
        nc.vector.memset(oneD_c[:].bitcast(f32), 1.0 / D)

        # ---------------- timestep/gripper embedding ----------------
        tstep = P.tile([1, 1], f32r, tag='tstep')
        nc.sync.dma_start(tstep[:], dram['tstep'][:])
        enc_b = P.tile([128, 12], f32, tag='enc_bias')
        nc.sync.dma_start(enc_b[:], dram['enc_bias'][:])
        tsin = P.tile([128, 3], f32r, tag='tsin')
        TWO_PI = float(2 * np.pi)
        PI = float(np.pi)
        C1 = 6.28125
        C2 = float(np.float32(2 * np.pi - C1))
        C3 = float(2 * np.pi - C1 - np.float32(2 * np.pi - C1))
        i32 = mybir.dt.int32
        for kt in range(NT):
            ps = psum_sm([128, 1])
            mmacc32(ps[:], [(tfreq[:, kt * 128:(kt + 1) * 128], tstep[:])])
            tv = TMPR.tile([128, 5], f32, tag='mcol', name='tv')
            ki = TMPR.tile([128, 1], i32, tag='mcoli', name='ki')
            xang, k4, kf, ycw, yw = (tv[:, 0:1], tv[:, 1:2], tv[:, 2:3],
                                     tv[:, 3:4], tv[:, 4:5])
            nc.vector.tensor_scalar_add(xang, ps[:], tphase[:, kt:kt + 1])
            nc.vector.tensor_scalar(k4, xang, 1.0 / TWO_PI, 0.5,
                                    ALU.mult, ALU.add)
            nc.vector.tensor_copy(ki[:], k4)
            nc.vector.tensor_copy(kf, ki[:])
            nc.vector.cody_waite_cascade(ycw, xang, kf, C1, C2, C3)
            nc.vector.add_range_wrap(yw, ycw, 0.0, PI, TWO_PI)
            nc.scalar.activation(tsin[:, kt:kt + 1], yw, AF.Sin)

        temb = P.tile([128, 3], f32r, tag='temb')
        st = P.tile([128, 3], f32r, tag='st')
        with tc.tile_pool(name='enc', bufs=1) as ENC:
            def vec_mlp(w1name, w2name, b_off, x_col, out_tile, nk=3):
                w1 = load_w(ENC, w1name, w1name)
                w2 = load_w(ENC, w2name, w2name)
                h = ENC.tile([128, 3], f32r, tag=f'h_{w1name}')
                for mt in range(NT):
                    ps = psum_sm([128, 1])
                    mmacc32(ps[:], [(w1[:, kk, mt * 128:(mt + 1) * 128],
                                     x_col[:, kk:kk + 1]) for kk in range(nk)])
                    nc.scalar.activation(h[:, mt:mt + 1], ps[:], AF.Relu,
                                         bias=enc_b[:, b_off + mt:b_off + mt + 1])
                for mt in range(NT):
                    ps = psum_sm([128, 1])
                    mmacc32(ps[:], [(w2[:, kk, mt * 128:(mt + 1) * 128],
                                     h[:, kk:kk + 1]) for kk in range(NT)])
                    nc.scalar.activation(out_tile[:, mt:mt + 1], ps[:],
                                         AF.Identity,
                                         bias=enc_b[:, b_off + 3 + mt:b_off + 4 + mt])

            tfe = ENC.tile([128, 3], f32r, tag='tfe')
            vec_mlp('te1', 'te2', 0, tsin, tfe)
            gflat = ENC.tile([128, 9], f32r, tag='gflat')
            nc.sync.dma_start(gflat[:],
                              dram['gflat'].rearrange('a p f -> p (a f)'))
            gfe = ENC.tile([128, 3], f32r, tag='gfe')
            vec_mlp('cg1', 'cg2', 6, gflat, gfe, nk=9)
            nc.vector.tensor_tensor(temb[:], tfe[:], gfe[:], ALU.add)
            sg = ENC.tile([128, 3], f32, tag='sg')
            nc.scalar.activation(sg[:], temb[:], AF.Sigmoid)
            nc.vector.tensor_tensor(st[:], temb[:], sg[:], ALU.mult)
        maybe_tap('temb', temb[:])

        ALL_BLOCKS = ['c0', 'c1', 's0', 's1', 's2', 's3', 'p0', 'p1']
        mods_all = P.tile([128, 12 * len(ALL_BLOCKS)], f32, tag='mods_all')
        with tc.tile_pool(name='adaw', bufs=2) as ADAW:
            for bi, nm in enumerate(ALL_BLOCKS):
                ada = load_w(ADAW, f'{nm}_ada', 'ada')
                adab = load_w(ADAW, f'{nm}_adab', 'adab')
                for mt in range(12):
                    ps = psum_sm([128, 1])
                    mmacc32(ps[:], [(ada[:, kk, mt * 128:(mt + 1) * 128],
                                     st[:, kk:kk + 1]) for kk in range(NT)])
                    nc.scalar.activation(
                        mods_all[:, bi * 12 + mt:bi * 12 + mt + 1], ps[:],
                        AF.Identity, bias=adab[:, mt:mt + 1])

        # ---------------- self/cross-q rope tables ----------------
        cos_s = P.tile([128, NTP, TS], f32, tag='cos_s')
        sin_s = P.tile([128, NTP, TS], f32, tag='sin_s')
        traj_xyz = P.tile([3, L], f32r, tag='traj_xyz')
        nc.sync.dma_start(traj_xyz[:], dram['traj_xyz'][:])
        for mt in range(NTP):
            ps = psum_mm([128, L])
            nc.tensor.matmul(ps[:], divp[:, mt * 128:(mt + 1) * 128],
                             traj_xyz[:], start=True, stop=True)
            for tab, ph in ((cos_s, float(np.pi / 2)), (sin_s, 0.0)):
                mt1 = TMPR.tile([128, 320], f32, tag='ropet1', name='mt1')
                nc.vector.add_range_wrap(mt1[:, 0:L], ps[:], ph, PI, TWO_PI)
                nc.scalar.activation(tab[:, mt, 0:L], mt1[:, 0:L], AF.Sin)
        nc.sync.dma_start(cos_s[:, :, L:TS],
                          dram['fps_cosT'].rearrange('a p f -> p a f')
                          .bitcast(f32))
        nc.sync.dma_start(sin_s[:, :, L:TS],
                          dram['fps_sinT'].rearrange('a p f -> p a f')
                          .bitcast(f32))

        # ---------------- trajectory encoder ----------------
        traj_aug = P.tile([8, L], f32r, tag='traj_aug')
        nc.sync.dma_start(traj_aug[:], dram['traj_aug'][:])
        traj_wT = P.tile([8, D], f32r, tag='traj_wT')
        nc.sync.dma_start(traj_wT[:], dram['traj_wT'][:])
        residA = P.tile([128, NT, TS], f32r, tag='residA')
        residB = P.tile([128, NT, TS], f32r, tag='residB')
        for mt in range(NT):
            ps = psum_mm([128, L])
            nc.tensor.matmul(ps[:], traj_wT[:, mt * 128:(mt + 1) * 128],
                             traj_aug[:], start=True, stop=True)
            nc.vector.tensor_tensor(residA[:, mt, 0:L], ps[:],
                                    sinembL[:, mt, :], ALU.add)
        maybe_tap('trajenc', residA[:, :, 0:L])

        # ---------------- shared block pieces ----------------
        def adaln_mods(nm, TMP):
            bi = ALL_BLOCKS.index(nm)
            mods = mods_all[:, bi * 12:(bi + 1) * 12]
            s1 = TMP.tile([128, 3], f32, tag='s1')
            nc.vector.tensor_scalar_add(s1[:], mods[:, 0:3], 1.0)
            s1f = TMP.tile([128, 3], f32, tag='s1f')
            nc.vector.tensor_scalar_add(s1f[:], mods[:, 6:9], 1.0)
            return mods, s1, s1f

        def layer_norm(x_in, x_out, bias, boff, T, TMP):
            ncs = _nc_chunks(T)
            stats = TMP.tile([1, 2 * T], f32, tag='lnstat', name='lnstat')
            work = TMP.tile([1, T], f32, tag='lnwork', name='lnwork')
            rr = TMP.tile([1, 2 * T], f32r, tag='lnrr', name='lnrr')
            m1, e1 = stats[:, 0:T], stats[:, T:2 * T]
            for (o, c) in ncs:
                ps = psum_sm([1, c])
                mmacc(ps[:], [(oneD_c[:], x_in[:, kt, o:o + c])
                              for kt in range(NT)])
                nc.vector.tensor_copy(m1[:, o:o + c], ps[:])
                ps2 = psum_sm([1, c])
                for kt in range(NT):
                    sqc = LNT.tile([128, 512], f32r, tag='lnsq', name='sqc')
                    nc.scalar.activation(sqc[:, 0:c], x_in[:, kt, o:o + c],
                                         AF.Square)
                    nc.tensor.matmul(ps2[:], oneD_c[:], sqc[:, 0:c],
                                     start=(kt == 0), stop=(kt == NT - 1))
                nc.vector.tensor_copy(e1[:, o:o + c], ps2[:])
            nc.vector.scalar_tensor_tensor(work[:], m1, 1.0, m1,
                                           ALU.mult, ALU.mult)
            nc.vector.tensor_tensor(e1, e1, work[:], ALU.subtract)
            nc.scalar.activation(work[:], e1, AF.Sqrt, bias=epsc[:])
            rstd, mr = rr[:, 0:T], rr[:, T:2 * T]
            nc.vector.reciprocal(rstd, work[:])
            nc.vector.tensor_tensor(mr, m1, rstd, ALU.mult)
            for (o, c) in ncs:
                psr = psum_mm([128, c])
                nc.tensor.matmul(psr[:], ones_r[:], rstd[:, o:o + c],
                                 start=True, stop=True)
                psm = psum_mm([128, c])
                nc.tensor.matmul(psm[:], ones_r[:], mr[:, o:o + c],
                                 start=True, stop=True)
                for kt in range(NT):
                    t1 = LNT.tile([128, 512], f32, tag='lnt1', name='lnt1')
                    nc.vector.tensor_tensor(t1[:, 0:c], x_in[:, kt, o:o + c],
                                            psr[:], ALU.mult)
                    nc.vector.tensor_tensor(t1[:, 0:c], t1[:, 0:c], psm[:],
                                            ALU.subtract)
                    nc.scalar.activation(
                        x_out[:, kt, o:o + c], t1[:, 0:c], AF.Identity,
                        bias=bias[:, boff + 3 + kt:boff + 4 + kt],
                        scale=bias[:, boff + kt:boff + 1 + kt])

        def attn_block(nm, x_resid, x_out, T, cross, WPb, TMP,
                       ctx_res=None):
            Tkv = N if cross else T
            bias = load_w(WPb, f'{nm}_bias', 'bias')
            wq = load_w(WPb, f'{nm}_wq', 'wq')
            wk = load_w(WPb, f'{nm}_wk', 'wk')
            wv = load_w(WPb, f'{nm}_wv', 'wv')
            wo = load_w(WPb, f'{nm}_wo', 'wo')
            w1 = load_w(WPb, f'{nm}_w1', 'w1')
            w2 = load_w(WPb, f'{nm}_w2', 'w2')
            bv = load_w(WPb, f'{nm}_bv', 'bv')
            mods, s1, s1f = adaln_mods(nm, TMP)

            ncq = _nc_chunks(T)
            nckv = _nc_chunks(Tkv)
            kvc = _kv_chunks(Tkv)
            nkt = NTP
            pdt = bf16
            Smat = S_pad_bf

            # adaln on q input
            aq = TMP.tile([128, NT, T], f32r, tag='gp1')
            for kt in range(NT):
                nc.scalar.activation(aq[:, kt, :], x_resid[:, kt, :],
                                     AF.Identity, bias=mods[:, 3 + kt:4 + kt],
                                     scale=s1[:, kt:kt + 1])

            # q projection (padded) + in-place rope
            qt = TMP.tile([128, NTP, T], pdt, tag='qt')
            for mt in range(NTP):
                for (o, c) in ncq:
                    ps = psum_mm([128, c])
                    mmacc(ps[:], [(wq[:, kt, mt * 128:(mt + 1) * 128],
                                   aq[:, kt, o:o + c]) for kt in range(NT)])
                    nc.scalar.activation(qt[:, mt, o:o + c], ps[:],
                                         AF.Identity, bias=bias[:, mt:mt + 1])
                for (o, c) in ncq:
                    pss = psum_mm([128, c])
                    nc.tensor.matmul(pss[:], Smat[:], qt[:, mt, o:o + c],
                                     start=True, stop=True)
                    t1 = TMPR.tile([128, 320], f32, tag='ropet1')
                    nc.vector.tensor_tensor(t1[:, 0:c], qt[:, mt, o:o + c],
                                            cos_s[:, mt, o:o + c], ALU.mult)
                    t2 = TMPR.tile([128, 320], f32, tag='ropet2')
                    nc.vector.tensor_tensor(t2[:, 0:c], pss[:],
                                            sin_s[:, mt, o:o + c], ALU.mult)
                    nc.vector.tensor_tensor(qt[:, mt, o:o + c], t1[:, 0:c],
                                            t2[:, 0:c], ALU.add)
            maybe_tap(f'{nm}_qrot', qt[:])

            # k projection + in-place rope
            if cross:
                ksrc, ctxT, ctx_xyz, krot, va = (
                    ctx_res['ctxT'], ctx_res['ctxT'], ctx_res['ctx_xyz'],
                    ctx_res['ctx_k'], ctx_res['ctx_va'])
            else:
                ksrc = x_resid
                krot = TMP.tile([128, NTP, T], bf16, tag='krot')
                va = TMP.tile([128, len(kvc), H, 65], bf16, tag='va')
            for mt in range(nkt):
                for (o, c) in nckv:
                    ps = psum_mm([128, c])
                    mmacc(ps[:], [(wk[:, kt, mt * 128:(mt + 1) * 128],
                                   ksrc[:, kt, o:o + c]) for kt in range(NT)])
                    nc.scalar.activation(krot[:, mt, o:o + c], ps[:],
                                         AF.Identity, bias=bias[:, 4 + mt:5 + mt])
                for (o, c) in nckv:
                    if cross:
                        psa = psum_mm([128, c])
                        nc.tensor.matmul(psa[:],
                                         divp[:, mt * 128:(mt + 1) * 128],
                                         ctx_xyz[:, o:o + c],
                                         start=True, stop=True)
                        cosk = ctx_res['CROPE'].tile([128, 320], f32,
                                                     tag='cosk', name='cosk')
                        nc.vector.add_range_wrap(cosk[:, 0:c], psa[:],
                                                 float(np.pi / 2), PI, TWO_PI)
                        nc.scalar.activation(cosk[:, 0:c], cosk[:, 0:c],
                                             AF.Sin)
                        sink = ctx_res['CROPE'].tile([128, 320], f32,
                                                     tag='sink', name='sink')
                        nc.vector.add_range_wrap(sink[:, 0:c], psa[:],
                                                 0.0, PI, TWO_PI)
                        nc.scalar.activation(sink[:, 0:c], sink[:, 0:c],
                                             AF.Sin)
                        cos_ap, sin_ap = cosk[:, 0:c], sink[:, 0:c]
                    else:
                        cos_ap = cos_s[:, mt, o:o + c]
                        sin_ap = sin_s[:, mt, o:o + c]
                    pss = psum_mm([128, c])
                    nc.tensor.matmul(pss[:], Smat[:], krot[:, mt, o:o + c],
                                     start=True, stop=True)
                    t1 = TMPR.tile([128, 320], f32, tag='ropet1')
                    nc.vector.tensor_tensor(t1[:, 0:c], krot[:, mt, o:o + c],
                                            cos_ap, ALU.mult)
                    t2 = TMPR.tile([128, 320], f32, tag='ropet2')
                    nc.vector.tensor_tensor(t2[:, 0:c], pss[:], sin_ap,
                                            ALU.mult)
                    nc.vector.tensor_tensor(krot[:, mt, o:o + c], t1[:, 0:c],
                                            t2[:, 0:c], ALU.add)
            maybe_tap(f'{nm}_krot', krot[:])

            # v projection (token-major) + ones column at 64
            nc.vector.memset(va[:, :, :, HD:65], 1.0)
            for ci, (o, c) in enumerate(kvc):
                ps = psum_mm([128, D])
                pairs = [(ksrc[:, kt, o:o + c], wv[:, kt, :])
                         for kt in range(NT)]
                pairs.append((ones_r[:, 0:c], bv[:]))
                mmacc(ps[0:c, :], pairs)
                nc.scalar.activation(
                    va[0:c, ci, :, 0:HD],
                    ps[0:c, :].rearrange('p (h d) -> p h d', h=H), AF.Copy)

            # attention (attout in padded head layout; pad rows are garbage
            # but multiply against zero rows of the padded Wo)
            attout = TMP.tile([128, NTP, T], f32r, tag='gp1', name='attout')
            nc.vector.memset(attout[32:64, :, :].bitcast(f32), 0.0)
            nc.vector.memset(attout[96:128, :, :].bitcast(f32), 0.0)
            for h in range(H):
                mt_q, off_q = h // 2, (h % 2) * 64
                for (oq, cq) in ncq:
                    Pb = PBP.tile([128, len(kvc), cq], pdt, tag='P')
                    for ci, (o, c) in enumerate(kvc):
                        ps = psum_mm([128, cq])
                        pairs = [(krot[off_q:off_q + HD, mt_q, o:o + c],
                                  qt[off_q:off_q + HD, mt_q, oq:oq + cq])]
                        mmacc(ps[0:c, :], pairs)
                        nc.scalar.activation(Pb[0:c, ci, 0:cq], ps[0:c, :],
                                             AF.Exp)
                    pav = psum_sm([65, cq])
                    mmacc(pav[:], [(va[0:c, ci, h, :], Pb[0:c, ci, 0:cq])
                                   for ci, (o, c) in enumerate(kvc)])
                    inv = TMPR.tile([1, 288], f32r, tag='inv')
                    nc.vector.reciprocal(inv[:, 0:cq], pav[64:65, :])
                    pb = psum_sm([HD, cq])
                    nc.tensor.matmul(pb[:], ones_r[:, 0:HD], inv[:, 0:cq],
                                     start=True, stop=True)
                    nc.scalar.activation(
                        attout[off_q:off_q + HD, mt_q, oq:oq + cq],
                        pav[0:HD, :], AF.Copy)
                    nc.vector.tensor_tensor(
                        attout[off_q:off_q + HD, mt_q, oq:oq + cq],
                        attout[off_q:off_q + HD, mt_q, oq:oq + cq],
                        pb[:], ALU.mult)
            maybe_tap(f'{nm}_attout', attout[:])

            # out projection + bias + residual
            x1 = TMP.tile([128, NT, T], f32r, tag='gp2')
            for mt in range(NT):
                for (o, c) in ncq:
                    ps = psum_mm([128, c])
                    mmacc(ps[:], [(wo[:, kt, mt * 128:(mt + 1) * 128],
                                   attout[:, kt, o:o + c])
                                  for kt in range(NTP)])
                    nc.vector.scalar_tensor_tensor(
                        x1[:, mt, o:o + c], ps[:], bias[:, 8 + mt:9 + mt],
                        x_resid[:, mt, o:o + c], ALU.add, ALU.add)

            xl = TMP.tile([128, NT, T], f32r, tag='xl')
            layer_norm(x1, xl, bias, 29, T, TMP)
            maybe_tap(f'{nm}_xl', xl[:])

            x2 = TMP.tile([128, NT, T], f32r, tag='gp1')
            for kt in range(NT):
                nc.scalar.activation(x2[:, kt, :], xl[:, kt, :], AF.Identity,
                                     bias=mods[:, 9 + kt:10 + kt],
                                     scale=s1f[:, kt:kt + 1])
            hbuf = TMP.tile([128, NT, T], f32r, tag='gp2')
            for mt in range(NT):
                for (o, c) in ncq:
                    ps = psum_mm([128, c])
                    mmacc(ps[:], [(w1[:, kt, mt * 128:(mt + 1) * 128],
                                   x2[:, kt, o:o + c]) for kt in range(NT)])
                    nc.scalar.activation(hbuf[:, mt, o:o + c], ps[:], AF.Relu,
                                         bias=bias[:, 11 + mt:12 + mt])
            x3 = TMP.tile([128, NT, T], f32r, tag='gp1')
            for mt in range(NT):
                for (o, c) in ncq:
                    ps = psum_mm([128, c])
                    mmacc(ps[:], [(w2[:, kt, mt * 128:(mt + 1) * 128],
                                   hbuf[:, kt, o:o + c]) for kt in range(NT)])
                    nc.vector.scalar_tensor_tensor(
                        x3[:, mt, o:o + c], ps[:], bias[:, 14 + mt:15 + mt],
                        xl[:, mt, o:o + c], ALU.add, ALU.add)
            layer_norm(x3, x_out, bias, 35, T, TMP)
            maybe_tap(f'{nm}_out', x_out[:, :, 0:T])

        # ---------------- cross phase ----------------
        with tc.tile_pool(name='wcross', bufs=1) as WPC, \
             tc.tile_pool(name='tmpc', bufs=1) as TMPC, \
             tc.tile_pool(name='crope', bufs=2) as CROPE, \
             tc.tile_pool(name='ctxp', bufs=1) as CP:
            ctx_res = {
                'CROPE': CROPE,
                'ctxT': CP.tile([128, NT, N], f32r, tag='ctxT',
                                name='ctxT'),
                'ctx_xyz': CP.tile([3, N], f32r, tag='ctx_xyz',
                                   name='ctx_xyz'),
                'ctx_k': CP.tile([128, NTP, N], bf16, tag='ctx_k',
                                 name='ctx_k'),
                'ctx_va': CP.tile([128, N // 128, H, 65], bf16,
                                  tag='ctx_va', name='ctx_va'),
            }
            nc.sync.dma_start(ctx_res['ctxT'][:],
                              dram['ctxT'].rearrange('a p f -> p a f'))
            nc.sync.dma_start(ctx_res['ctx_xyz'][:], dram['ctx_xyz'][:])
            attn_block('c0', residA[:, :, 0:L], residB[:, :, 0:L], L,
                       True, WPC, TMPC, ctx_res)
            attn_block('c1', residB[:, :, 0:L], residA[:, :, 0:L], L,
                       True, WPC, TMPC, ctx_res)

        # feats assembly: cross output already in residA cols 0:L
        nc.sync.dma_start(residA[:, :, L:TS],
                          dram['fpsT'].rearrange('a p f -> p a f'))
        maybe_tap('feats', residA[:])

        # ---------------- self phase + heads ----------------
        with tc.tile_pool(name='wself', bufs=2) as WPS, \
             tc.tile_pool(name='tmps', bufs=1) as TMPS:
            cur, nxt = residA, residB
            for nm in ('s0', 's1', 's2', 's3', 'p0', 'p1'):
                attn_block(nm, cur, nxt, TS, False, WPS, TMPS)
                cur, nxt = nxt, cur

            head_b = P.tile([128, 9], f32, tag='head_bias')
            nc.sync.dma_start(head_b[:], dram['head_bias'][:])
            posf = cur

            hw_tags = {'pproj': 'wv', 'pp1': 'wo', 'op1': 'w1'}

            def head_proj(wname, boff, src, func, Tsrc):
                wt = load_w(WPS, wname, hw_tags[wname])
                out = TMPS.tile([128, NT, L], f32r, tag=f'h_{wname}')
                for mt in range(NT):
                    ps = psum_mm([128, L])
                    mmacc(ps[:], [(wt[:, kt, mt * 128:(mt + 1) * 128],
                                   src[:, kt, 0:L]) for kt in range(NT)])
                    nc.scalar.activation(out[:, mt, :], ps[:], func,
                                         bias=head_b[:, boff + mt:boff + mt + 1])
                return out

            pf = head_proj('pproj', 0, posf, AF.Identity, TS)
            maybe_tap('posf', pf[:])
            h1 = head_proj('pp1', 3, pf, AF.Relu, L)
            h2 = head_proj('op1', 6, pf, AF.Relu, L)
            pp2T = load_w(WPS, 'pp2T', 'w2')
            op2T = load_w(WPS, 'op2T', 'wk')
            pp2b = load_w(WPS, 'pp2_b', 'bv')
            op2b = load_w(WPS, 'op2_b', 'bv')
            outsb = TMPS.tile([L, 8], f32, tag='outsb')
            ps = psum_sm([L, 7])
            mmacc32(ps[:], [(h1[:, kt, :], pp2T[:, kt, :])
                            for kt in range(NT)]
                    + [(ones_r[:, 0:L], pp2b[:])])
            nc.scalar.copy(outsb[:, 0:7], ps[:])
            ps2 = psum_sm([L, 1])
            mmacc32(ps2[:], [(h2[:, kt, :], op2T[:, kt, :])
                             for kt in range(NT)]
                     + [(ones_r[:, 0:L], op2b[:])])
            nc.scalar.copy(outsb[:, 7:8], ps2[:])
            nc.sync.dma_start(dram['out'][:], outsb[:])

    nc.compile()
    bd.dram = dram
    return bd


# ------------------------------------------------------------------- entry
_CACHE = {}


def kernel(**inputs):
    key = 'bd' + ','.join(sorted(DEBUG_TAPS))
    if key not in _CACHE:
        _CACHE[key] = build(tuple(DEBUG_TAPS))
    bd = _CACHE[key]
    w = prep_weights(inputs['params'])
    in_maps = []
    for b in range(N_CORES):
        m = dict(w)
        m.update(prep_core_inputs(b, inputs))
        in_maps.append(m)
    res = run_bass_kernel_spmd(bd.nc, in_maps, core_ids=list(range(N_CORES)))
    _CACHE['last_results'] = res
    out = np.stack([res.results[b]['out'] for b in range(N_CORES)], axis=0)
    return out.astype(np.float32)


# revision 30
# speedup vs baseline: 1.1017x; 1.0455x over previous
"""Trainium2 Bass kernel for nn_DiffuserJointer (dense diffusion transformer).

Strategy: data-parallel over batch B=8 across 8 NeuronCores (one batch
element per core; no collectives). On-core layout is feature-major
(features on partitions, tokens on the free dim) so every matmul contracts
over partitions with no transposes. All matmuls run in float32r (fp32 bits,
~tf32 precision, bf16-rate on the PE). Attention uses a transposed-scores
formulation (kv on partitions) so softmax normalization folds into an
augmented ones-column of V; RoPE's pair-shuffle is a constant 128x128
matmul; per-token normalizers broadcast via rank-1 matmuls.
"""
import sys
sys.path.insert(0, '/opt/trn_rl_repo')

import numpy as np
import ml_dtypes

import concourse.bass as bass
import concourse.bacc as bacc
import concourse.mybir as mybir
import concourse.tile as tile
from concourse.bass_utils import run_bass_kernel_spmd

f32 = mybir.dt.float32
f32r = mybir.dt.float32r
bf16 = mybir.dt.bfloat16
AF = mybir.ActivationFunctionType
ALU = mybir.AluOpType

# ---- problem dims (hardcoded) ----
D, H, B, L, N, K, NHIST = 384, 8, 8, 64, 2560, 512, 3
HD = D // H            # 48
DP = 512               # padded q/k feature dim (64 per head)
TS = L + K             # 576 self-attn tokens
NT = 3                 # feature tiles of 128
NTP = 4                # padded feature tiles
N_CORES = 8
EPS = 1e-5

# optional debug taps: list of names filled by the test harness before build
DEBUG_TAPS = []
_TAP_SHAPES = {}


# ---------------------------------------------------------------- host prep
def _sin_emb_table(x, dim):
    half = dim // 2
    freqs = np.exp(-np.log(10000.0) / (half - 1) * np.arange(half, dtype=np.float64))
    ang = np.asarray(x, np.float64)[:, None] * freqs[None, :]
    return np.concatenate([np.sin(ang), np.cos(ang)], axis=-1).astype(np.float32)


def _rope_div():
    d = D // 3  # 128
    return np.exp(np.arange(0, d, 2, dtype=np.float64)
                  * (-np.log(10000.0) / d)).astype(np.float32)


def _pad_rows(w):
    """(384, ...) head rows -> (512, ...): head h rows h*48..+47 -> h*64..+47."""
    out = np.zeros((DP,) + w.shape[1:], dtype=np.float32)
    for h in range(H):
        out[h * 64:h * 64 + HD] = w[h * HD:(h + 1) * HD]
    return out


def _tile_lhsT(wT):
    din = wT.shape[0]
    assert din % 128 == 0
    return np.ascontiguousarray(wT.reshape(din // 128, 128, *wT.shape[1:]))


def _cols(v):
    return np.ascontiguousarray(np.asarray(v, np.float32).reshape(NT, 128).T)


def _cols_p(v):
    return np.ascontiguousarray(np.asarray(v, np.float32).reshape(NTP, 128).T)


def _shuffle_mat(padded):
    S = np.zeros((128, 128), dtype=np.float32)
    if padded:
        for b0 in (0, 64):
            for i in range(0, HD, 2):
                S[b0 + i, b0 + i + 1] = -1.0
                S[b0 + i + 1, b0 + i] = 1.0
    else:
        for i in range(0, 128, 2):
            S[i, i + 1] = -1.0
            S[i + 1, i] = 1.0
    return np.ascontiguousarray(S.T)


def _divmat(padded):
    div = _rope_div()
    n = DP if padded else D
    M = np.zeros((3, n), dtype=np.float32)
    for r in range(n):
        if padded:
            h, j = r // 64, r % 64
            if j >= HD:
                continue
            d = h * HD + j
        else:
            d = r
        M[d // 128, r] = div[(d % 128) // 2]
    return M


def _pad_feat_T(x):
    """(T, 384) -> (512, T) padded feature-major."""
    xT = np.zeros((DP, x.shape[0]), dtype=np.float32)
    for h in range(H):
        xT[h * 64:h * 64 + HD] = x[:, h * HD:(h + 1) * HD].T
    return np.ascontiguousarray(xT)


def prep_weights(params):
    p = {k: (np.asarray(v, np.float32) if not isinstance(v, (list, dict)) else v)
         for k, v in params.items()}
    w = {}

    def blk(name, bp, cross):
        bp = {k: np.asarray(v, np.float32) for k, v in bp.items()}
        wq, wk, wv = bp['in_w'][:D], bp['in_w'][D:2 * D], bp['in_w'][2 * D:]
        bq, bk, bv = bp['in_b'][:D], bp['in_b'][D:2 * D], bp['in_b'][2 * D:]
        sc = HD ** -0.5
        bf = ml_dtypes.bfloat16
        wq_p, bq_p = _pad_rows(wq * sc), _pad_rows(bq * sc)
        w[f'{name}_wq'] = _tile_lhsT(wq_p.T.copy()).astype(bf)
        wk_t = _tile_lhsT(_pad_rows(wk).T.copy())
        w[f'{name}_wk'] = wk_t if cross else wk_t.astype(bf)
        bk_cols = _cols_p(_pad_rows(bk))
        wv_t = _tile_lhsT(wv.T.copy())
        w[f'{name}_wv'] = wv_t if cross else wv_t.astype(bf)
        w[f'{name}_wo'] = _tile_lhsT(_pad_rows(bp['out_w'].T.copy())).astype(bf)
        w[f'{name}_w1'] = _tile_lhsT(bp['w1'].T.copy()).astype(bf)
        w[f'{name}_w2'] = _tile_lhsT(bp['w2'].T.copy()).astype(bf)
        ada_cat = np.concatenate([bp['ada_w'], bp['f_ada_w']], axis=0)
        w[f'{name}_ada'] = _tile_lhsT(ada_cat.T.copy())       # (3,128,1536)
        cols = [
            _cols_p(bq_p),                               # 0:4
            bk_cols,                                     # 4:8
            _cols(bp['out_b']),                          # 8:11
            _cols(bp['b1']),                             # 11:14
            _cols(bp['b2']),                             # 14:17
            np.ascontiguousarray(np.concatenate(
                [bp['ada_b'], bp['f_ada_b']]).reshape(12, 128).T),  # 17:29
            _cols(bp['ln1_g']), _cols(bp['ln1_b']),      # 29:32, 32:35
            _cols(bp['ln2_g']), _cols(bp['ln2_b']),      # 35:38, 38:41
        ]
        w[f'{name}_bias'] = np.ascontiguousarray(np.concatenate(cols, axis=1))
        bv_t = np.ascontiguousarray(bv.reshape(1, D))
        w[f'{name}_bv'] = bv_t if cross else bv_t.astype(bf)
        w[f'{name}_adab'] = np.ascontiguousarray(np.concatenate(
            [bp['ada_b'], bp['f_ada_b']]).reshape(12, 128).T)

    for i, bp in enumerate(p['cross']):
        blk(f'c{i}', bp, cross=True)
    for i, bp in enumerate(p['selfa']):
        blk(f's{i}', bp, cross=False)
    for i, bp in enumerate(p['posa']):
        blk(f'p{i}', bp, cross=False)

    traj_aug = np.concatenate([p['traj_w'], p['traj_b'][:, None]], axis=1)
    w['traj_wT'] = np.ascontiguousarray(traj_aug.T)     # (8, 384)
    w['te1'] = _tile_lhsT(p['te1_w'].T.copy())
    w['te2'] = _tile_lhsT(p['te2_w'].T.copy())
    w['cg1'] = _tile_lhsT(p['cg1_w'].T.copy())          # (9,128,384)
    w['cg2'] = _tile_lhsT(p['cg2_w'].T.copy())
    w['enc_bias'] = np.ascontiguousarray(np.concatenate(
        [_cols(p['te1_b']), _cols(p['te2_b']), _cols(p['cg1_b']),
         _cols(p['cg2_b'])], axis=1))                   # (128, 12)
    w['pproj'] = _tile_lhsT(p['pproj_w'].T.copy())
    w['pp1'] = _tile_lhsT(p['pp1_w'].T.copy())
    w['op1'] = _tile_lhsT(p['op1_w'].T.copy())
    w['head_bias'] = np.ascontiguousarray(np.concatenate(
        [_cols(p['pproj_b']), _cols(p['pp1_b']), _cols(p['op1_b'])], axis=1))
    w['pp2T'] = _tile_lhsT(p['pp2_w'].T.copy())         # (3,128,7)
    w['op2T'] = _tile_lhsT(p['op2_w'].T.copy())         # (3,128,1)
    w['pp2_b'] = np.ascontiguousarray(p['pp2_b'].reshape(1, 7))
    w['op2_b'] = np.ascontiguousarray(p['op2_b'].reshape(1, 1))

    w['S_pad'] = _shuffle_mat(True)
    w['S_nop'] = _shuffle_mat(False)
    w['divmat_p'] = _divmat(True)                        # (3,512)
    w['divmat_n'] = _divmat(False)                       # (3,384)
    w['sinembL'] = _tile_lhsT(np.ascontiguousarray(
        _sin_emb_table(np.arange(L), D).T))              # (3,128,64)
    half = D // 2
    tf = np.exp(-np.log(10000.0) / (half - 1)
                * np.arange(half)).astype(np.float32)
    w['tfreq'] = np.ascontiguousarray(np.concatenate([tf, tf]).reshape(1, D))
    phase = np.zeros(D, np.float32)
    phase[half:] = np.pi / 2
    w['tphase'] = _cols(phase)                           # (128,3)
    return w


def prep_core_inputs(b, inputs):
    m = {}
    traj = np.asarray(inputs['trajectory'], np.float32)[b]          # (64,7)
    m['traj_aug'] = np.ascontiguousarray(
        np.concatenate([traj.T, np.ones((1, L), np.float32)], axis=0))
    m['traj_xyz'] = np.ascontiguousarray(traj[:, :3].T)             # (3,64)
    m['tstep'] = np.array([[float(np.asarray(inputs['timestep'])[b])]],
                          np.float32)
    m['ctxT'] = np.ascontiguousarray(
        np.asarray(inputs['context_feats'], np.float32)[b].T
        .reshape(NT, 128, N))
    m['ctx_xyz'] = np.ascontiguousarray(
        np.asarray(inputs['context'], np.float32)[b].T)             # (3,2560)
    m['gflat'] = np.ascontiguousarray(
        np.asarray(inputs['adaln_gripper_feats'], np.float32)[b]
        .reshape(-1).reshape(9, 128, 1))
    m['fpsT'] = np.ascontiguousarray(
        np.asarray(inputs['fps_feats'], np.float32)[:, b, :].T
        .reshape(NT, 128, K))
    fp = np.asarray(inputs['fps_pos'], np.float32)[b]               # (512,384,2)
    m['fps_cosT'] = np.ascontiguousarray(
        _pad_feat_T(fp[:, :, 0]).reshape(NTP, 128, K))
    m['fps_sinT'] = np.ascontiguousarray(
        _pad_feat_T(fp[:, :, 1]).reshape(NTP, 128, K))
    return m


# ------------------------------------------------------------- bass builder
def _nc_chunks(T):
    if T <= 512:
        return [(0, T)]
    if T == TS:
        return [(0, 288), (288, 288)]
    return [(i, 320) for i in range(0, T, 320)]


def _kv_chunks(T):
    out, i = [], 0
    while i < T:
        c = min(128, T - i)
        out.append((i, c))
        i += c
    return out


def _head_rows(h, padded):
    if padded:
        return [(h // 2, (h % 2) * 64, HD)]
    lo, hi = h * HD, (h + 1) * HD
    out = []
    for t in range(lo // 128, (hi - 1) // 128 + 1):
        s = max(lo, t * 128) - t * 128
        e = min(hi, (t + 1) * 128) - t * 128
        out.append((t, s, e - s))
    return out


class _BD:
    pass


def build(debug_taps=()):
    bd = _BD()
    nc = bacc.Bacc('TRN2', target_bir_lowering=False, debug=False,
                   num_devices=N_CORES)
    bd.nc = nc
    dram = {}

    def din(name, shape, dtype=f32r):
        dram[name] = nc.dram_tensor(name, list(shape), dtype,
                                    kind='ExternalInput').ap()

    def dout(name, shape, dtype=f32):
        dram[name] = nc.dram_tensor(name, list(shape), dtype,
                                    kind='ExternalOutput').ap()

    din('traj_aug', (8, L))
    din('traj_xyz', (3, L))
    din('tstep', (1, 1))
    din('ctxT', (NT, 128, N))
    din('ctx_xyz', (3, N))
    din('gflat', (9, 128, 1))
    din('fpsT', (NT, 128, K))
    din('fps_cosT', (NTP, 128, K))
    din('fps_sinT', (NTP, 128, K))
    blocks = ['c0', 'c1', 's0', 's1', 's2', 's3', 'p0', 'p1']
    for nm in blocks:
        cross = nm.startswith('c')
        din(f'{nm}_wq', (NT, 128, DP), bf16)
        din(f'{nm}_wk', (NT, 128, DP), f32r if cross else bf16)
        din(f'{nm}_wv', (NT, 128, D), f32r if cross else bf16)
        din(f'{nm}_wo', (NTP, 128, D), bf16)
        din(f'{nm}_w1', (NT, 128, D), bf16)
        din(f'{nm}_w2', (NT, 128, D), bf16)
        din(f'{nm}_ada', (NT, 128, 4 * D))
        din(f'{nm}_bias', (128, 41), f32)
        din(f'{nm}_adab', (128, 12), f32)
        din(f'{nm}_bv', (1, D), f32r if cross else bf16)
    din('traj_wT', (8, D))
    for nm2 in ('te1', 'te2', 'cg2', 'pproj', 'pp1', 'op1'):
        din(nm2, (NT, 128, D))
    din('cg1', (9, 128, D))
    din('enc_bias', (128, 12), f32)
    din('head_bias', (128, 9), f32)
    din('pp2T', (NT, 128, 7))
    din('op2T', (NT, 128, 1))
    din('pp2_b', (1, 7))
    din('op2_b', (1, 1))
    din('S_pad', (128, 128))
    din('S_nop', (128, 128))
    din('divmat_p', (3, DP))
    din('divmat_n', (3, D))
    din('sinembL', (NT, 128, L), f32)
    din('tfreq', (1, D))
    din('tphase', (128, 3), f32)
    dout('out', (L, 8))
    for tname in debug_taps:
        dout(f'tap_{tname}', _TAP_SHAPES[tname])

    def maybe_tap(name, ap):
        if name in debug_taps:
            nc.sync.dma_start(dram[f'tap_{name}'][:].bitcast(ap.dtype), ap)

    with tile.TileContext(nc) as tc, \
         nc.allow_low_precision(reason='f32r is full fp32 bits'), \
         tc.tile_pool(name='persist', bufs=1) as P, \
         tc.tile_pool(name='psA', bufs=5, space='PSUM') as PSA, \
         tc.tile_pool(name='psB', bufs=3, space='PSUM') as PSB, \
         tc.tile_pool(name='roptmp', bufs=2) as TMPR, \
         tc.tile_pool(name='pbuf', bufs=2) as PBP, \
         tc.tile_pool(name='lnt', bufs=2) as LNT:

        cnt = [0]

        def psum_mm(shape):
            cnt[0] += 1
            return PSA.tile(shape, f32, tag='mm', name=f'mm{cnt[0]}')

        def psum_sm(shape):
            cnt[0] += 1
            return PSB.tile(shape, f32, tag='sm', name=f'sm{cnt[0]}')

        def mmacc(ps, pairs):
            n = len(pairs)
            for i, (lhsT, rhs) in enumerate(pairs):
                nc.tensor.matmul(ps, lhsT, rhs, start=(i == 0),
                                 stop=(i == n - 1))

        def f32cast(ap):
            return ap.bitcast(f32) if ap.dtype == f32r else ap

        def mmacc32(ps, pairs):
            n = len(pairs)
            for i, (lhsT, rhs) in enumerate(pairs):
                nc.tensor.matmul(ps, f32cast(lhsT), f32cast(rhs),
                                 start=(i == 0), stop=(i == n - 1))

        def load_w(pool, name, tag):
            d = dram[name]
            sh = list(d.tensor.shape)
            if len(sh) == 3:
                t = pool.tile([128, sh[0], sh[2]], d.tensor.dtype, tag=tag)
                nc.sync.dma_start(t[:], d.rearrange('a p f -> p a f'))
            else:
                t = pool.tile(sh, d.tensor.dtype, tag=tag)
                nc.sync.dma_start(t[:], d[:])
            return t

        # ---------------- constants ----------------
        S_pad = P.tile([128, 128], f32r, tag='S_pad')
        nc.sync.dma_start(S_pad[:], dram['S_pad'][:])
        S_pad_bf = P.tile([128, 128], bf16, tag='S_pad_bf')
        nc.vector.tensor_copy(S_pad_bf[:], S_pad[:])
        ones_r_bf = P.tile([1, 128], bf16, tag='ones_r_bf')
        nc.vector.memset(ones_r_bf[:], 1.0)
        divp = P.tile([3, DP], f32r, tag='divp')
        nc.sync.dma_start(divp[:], dram['divmat_p'][:])
        sinembL = P.tile([128, NT, L], f32, tag='sinembL')
        nc.sync.dma_start(sinembL[:], dram['sinembL'].rearrange('a p f -> p a f'))
        tfreq = P.tile([1, D], f32r, tag='tfreq')
        nc.sync.dma_start(tfreq[:], dram['tfreq'][:])
        tphase = P.tile([128, 3], f32, tag='tphase')
        nc.sync.dma_start(tphase[:], dram['tphase'][:])
        ones_r = P.tile([1, 128], f32r, tag='ones_r')
        nc.vector.memset(ones_r[:].bitcast(f32), 1.0)
        ones_c = P.tile([128, 1], f32r, tag='ones_c')
        nc.vector.memset(ones_c[:].bitcast(f32), 1.0)
        halfpi = P.tile([128, 1], f32, tag='halfpi')
        nc.vector.memset(halfpi[:], float(np.pi / 2))
        epsc = P.tile([1, 1], f32, tag='epsc')
        nc.vector.memset(epsc[:], EPS)
        oneD_c = P.tile([128, 1], f32r, tag='oneD_c')
        nc.vector.memset(oneD_c[:].bitcast(f32), 1.0 / D)

        # ---------------- timestep/gripper embedding ----------------
        tstep = P.tile([1, 1], f32r, tag='tstep')
        nc.sync.dma_start(tstep[:], dram['tstep'][:])
        enc_b = P.tile([128, 12], f32, tag='enc_bias')
        nc.sync.dma_start(enc_b[:], dram['enc_bias'][:])
        tsin = P.tile([128, 3], f32r, tag='tsin')
        TWO_PI = float(2 * np.pi)
        PI = float(np.pi)
        C1 = 6.28125
        C2 = float(np.float32(2 * np.pi - C1))
        C3 = float(2 * np.pi - C1 - np.float32(2 * np.pi - C1))
        i32 = mybir.dt.int32
        for kt in range(NT):
            ps = psum_sm([128, 1])
            mmacc32(ps[:], [(tfreq[:, kt * 128:(kt + 1) * 128], tstep[:])])
            tv = TMPR.tile([128, 5], f32, tag='mcol', name='tv')
            ki = TMPR.tile([128, 1], i32, tag='mcoli', name='ki')
            xang, k4, kf, ycw, yw = (tv[:, 0:1], tv[:, 1:2], tv[:, 2:3],
                                     tv[:, 3:4], tv[:, 4:5])
            nc.vector.tensor_scalar_add(xang, ps[:], tphase[:, kt:kt + 1])
            nc.vector.tensor_scalar(k4, xang, 1.0 / TWO_PI, 0.5,
                                    ALU.mult, ALU.add)
            nc.vector.tensor_copy(ki[:], k4)
            nc.vector.tensor_copy(kf, ki[:])
            nc.vector.cody_waite_cascade(ycw, xang, kf, C1, C2, C3)
            nc.vector.add_range_wrap(yw, ycw, 0.0, PI, TWO_PI)
            nc.scalar.activation(tsin[:, kt:kt + 1], yw, AF.Sin)

        temb = P.tile([128, 3], f32r, tag='temb')
        st = P.tile([128, 3], f32r, tag='st')
        with tc.tile_pool(name='enc', bufs=1) as ENC:
            def vec_mlp(w1name, w2name, b_off, x_col, out_tile, nk=3):
                w1 = load_w(ENC, w1name, w1name)
                w2 = load_w(ENC, w2name, w2name)
                h = ENC.tile([128, 3], f32r, tag=f'h_{w1name}')
                for mt in range(NT):
                    ps = psum_sm([128, 1])
                    mmacc32(ps[:], [(w1[:, kk, mt * 128:(mt + 1) * 128],
                                     x_col[:, kk:kk + 1]) for kk in range(nk)])
                    nc.scalar.activation(h[:, mt:mt + 1], ps[:], AF.Relu,
                                         bias=enc_b[:, b_off + mt:b_off + mt + 1])
                for mt in range(NT):
                    ps = psum_sm([128, 1])
                    mmacc32(ps[:], [(w2[:, kk, mt * 128:(mt + 1) * 128],
                                     h[:, kk:kk + 1]) for kk in range(NT)])
                    nc.scalar.activation(out_tile[:, mt:mt + 1], ps[:],
                                         AF.Identity,
                                         bias=enc_b[:, b_off + 3 + mt:b_off + 4 + mt])

            tfe = ENC.tile([128, 3], f32r, tag='tfe')
            vec_mlp('te1', 'te2', 0, tsin, tfe)
            gflat = ENC.tile([128, 9], f32r, tag='gflat')
            nc.sync.dma_start(gflat[:],
                              dram['gflat'].rearrange('a p f -> p (a f)'))
            gfe = ENC.tile([128, 3], f32r, tag='gfe')
            vec_mlp('cg1', 'cg2', 6, gflat, gfe, nk=9)
            nc.vector.tensor_tensor(temb[:], tfe[:], gfe[:], ALU.add)
            sg = ENC.tile([128, 3], f32, tag='sg')
            nc.scalar.activation(sg[:], temb[:], AF.Sigmoid)
            nc.vector.tensor_tensor(st[:], temb[:], sg[:], ALU.mult)
        maybe_tap('temb', temb[:])

        ALL_BLOCKS = ['c0', 'c1', 's0', 's1', 's2', 's3', 'p0', 'p1']
        mods_all = P.tile([128, 12 * len(ALL_BLOCKS)], f32, tag='mods_all')
        with tc.tile_pool(name='adaw', bufs=2) as ADAW:
            for bi, nm in enumerate(ALL_BLOCKS):
                ada = load_w(ADAW, f'{nm}_ada', 'ada')
                adab = load_w(ADAW, f'{nm}_adab', 'adab')
                for mt in range(12):
                    ps = psum_sm([128, 1])
                    mmacc32(ps[:], [(ada[:, kk, mt * 128:(mt + 1) * 128],
                                     st[:, kk:kk + 1]) for kk in range(NT)])
                    nc.scalar.activation(
                        mods_all[:, bi * 12 + mt:bi * 12 + mt + 1], ps[:],
                        AF.Identity, bias=adab[:, mt:mt + 1])

        # ---------------- self/cross-q rope tables ----------------
        cos_s = P.tile([128, NTP, TS], f32, tag='cos_s')
        sin_s = P.tile([128, NTP, TS], f32, tag='sin_s')
        traj_xyz = P.tile([3, L], f32r, tag='traj_xyz')
        nc.sync.dma_start(traj_xyz[:], dram['traj_xyz'][:])
        for mt in range(NTP):
            ps = psum_mm([128, L])
            nc.tensor.matmul(ps[:], divp[:, mt * 128:(mt + 1) * 128],
                             traj_xyz[:], start=True, stop=True)
            for tab, ph in ((cos_s, float(np.pi / 2)), (sin_s, 0.0)):
                mt1 = TMPR.tile([128, 320], f32, tag='ropet1', name='mt1')
                nc.vector.add_range_wrap(mt1[:, 0:L], ps[:], ph, PI, TWO_PI)
                nc.scalar.activation(tab[:, mt, 0:L], mt1[:, 0:L], AF.Sin)
        nc.sync.dma_start(cos_s[:, :, L:TS],
                          dram['fps_cosT'].rearrange('a p f -> p a f')
                          .bitcast(f32))
        nc.sync.dma_start(sin_s[:, :, L:TS],
                          dram['fps_sinT'].rearrange('a p f -> p a f')
                          .bitcast(f32))

        # ---------------- trajectory encoder ----------------
        traj_aug = P.tile([8, L], f32r, tag='traj_aug')
        nc.sync.dma_start(traj_aug[:], dram['traj_aug'][:])
        traj_wT = P.tile([8, D], f32r, tag='traj_wT')
        nc.sync.dma_start(traj_wT[:], dram['traj_wT'][:])
        residA = P.tile([128, NT, TS], f32r, tag='residA')
        residB = P.tile([128, NT, TS], f32r, tag='residB')
        for mt in range(NT):
            ps = psum_mm([128, L])
            nc.tensor.matmul(ps[:], traj_wT[:, mt * 128:(mt + 1) * 128],
                             traj_aug[:], start=True, stop=True)
            nc.vector.tensor_tensor(residA[:, mt, 0:L], ps[:],
                                    sinembL[:, mt, :], ALU.add)
        maybe_tap('trajenc', residA[:, :, 0:L])

        # ---------------- shared block pieces ----------------
        def adaln_mods(nm, TMP):
            bi = ALL_BLOCKS.index(nm)
            mods = mods_all[:, bi * 12:(bi + 1) * 12]
            s1 = TMP.tile([128, 3], f32, tag='s1')
            nc.vector.tensor_scalar_add(s1[:], mods[:, 0:3], 1.0)
            s1f = TMP.tile([128, 3], f32, tag='s1f')
            nc.vector.tensor_scalar_add(s1f[:], mods[:, 6:9], 1.0)
            return mods, s1, s1f

        def layer_norm(x_in, x_out, bias, boff, T, TMP):
            ncs = _nc_chunks(T)
            stats = TMP.tile([1, 2 * T], f32, tag='lnstat', name='lnstat')
            work = TMP.tile([1, T], f32, tag='lnwork', name='lnwork')
            rr = TMP.tile([1, 2 * T], f32r, tag='lnrr', name='lnrr')
            m1, e1 = stats[:, 0:T], stats[:, T:2 * T]
            for (o, c) in ncs:
                ps = psum_sm([1, c])
                mmacc(ps[:], [(oneD_c[:], x_in[:, kt, o:o + c])
                              for kt in range(NT)])
                nc.vector.tensor_copy(m1[:, o:o + c], ps[:])
                ps2 = psum_sm([1, c])
                for kt in range(NT):
                    sqc = LNT.tile([128, 512], f32r, tag='lnsq', name='sqc')
                    nc.scalar.activation(sqc[:, 0:c], x_in[:, kt, o:o + c],
                                         AF.Square)
                    nc.tensor.matmul(ps2[:], oneD_c[:], sqc[:, 0:c],
                                     start=(kt == 0), stop=(kt == NT - 1))
                nc.vector.tensor_copy(e1[:, o:o + c], ps2[:])
            nc.vector.scalar_tensor_tensor(work[:], m1, 1.0, m1,
                                           ALU.mult, ALU.mult)
            nc.vector.tensor_tensor(e1, e1, work[:], ALU.subtract)
            nc.scalar.activation(work[:], e1, AF.Sqrt, bias=epsc[:])
            rstd, mr = rr[:, 0:T], rr[:, T:2 * T]
            nc.vector.reciprocal(rstd, work[:])
            nc.vector.tensor_tensor(mr, m1, rstd, ALU.mult)
            for (o, c) in ncs:
                psr = psum_mm([128, c])
                nc.tensor.matmul(psr[:], ones_r[:], rstd[:, o:o + c],
                                 start=True, stop=True)
                psm = psum_mm([128, c])
                nc.tensor.matmul(psm[:], ones_r[:], mr[:, o:o + c],
                                 start=True, stop=True)
                for kt in range(NT):
                    t1 = LNT.tile([128, 512], f32, tag='lnt1', name='lnt1')
                    nc.vector.tensor_tensor(t1[:, 0:c], x_in[:, kt, o:o + c],
                                            psr[:], ALU.mult)
                    nc.vector.tensor_tensor(t1[:, 0:c], t1[:, 0:c], psm[:],
                                            ALU.subtract)
                    nc.scalar.activation(
                        x_out[:, kt, o:o + c], t1[:, 0:c], AF.Identity,
                        bias=bias[:, boff + 3 + kt:boff + 4 + kt],
                        scale=bias[:, boff + kt:boff + 1 + kt])

        def attn_block(nm, x_resid, x_out, T, cross, WPb, TMP,
                       ctx_res=None):
            Tkv = N if cross else T
            bias = load_w(WPb, f'{nm}_bias', 'bias')
            wq = load_w(WPb, f'{nm}_wq', 'wq')
            wk = load_w(WPb, f'{nm}_wk', 'wk')
            wv = load_w(WPb, f'{nm}_wv', 'wv')
            wo = load_w(WPb, f'{nm}_wo', 'wo')
            w1 = load_w(WPb, f'{nm}_w1', 'w1')
            w2 = load_w(WPb, f'{nm}_w2', 'w2')
            bv = load_w(WPb, f'{nm}_bv', 'bv')
            mods, s1, s1f = adaln_mods(nm, TMP)

            ncq = _nc_chunks(T)
            nckv = _nc_chunks(Tkv)
            kvc = _kv_chunks(Tkv)
            nkt = NTP
            pdt = bf16
            Smat = S_pad_bf

            # adaln on q input
            aq = TMP.tile([128, NT, T], bf16, tag='gp1')
            for kt in range(NT):
                nc.scalar.activation(aq[:, kt, :], x_resid[:, kt, :],
                                     AF.Identity, bias=mods[:, 3 + kt:4 + kt],
                                     scale=s1[:, kt:kt + 1])

            # q projection (padded) + in-place rope
            qt = TMP.tile([128, NTP, T], pdt, tag='qt')
            for mt in range(NTP):
                for (o, c) in ncq:
                    ps = psum_mm([128, c])
                    mmacc(ps[:], [(wq[:, kt, mt * 128:(mt + 1) * 128],
                                   aq[:, kt, o:o + c]) for kt in range(NT)])
                    nc.scalar.activation(qt[:, mt, o:o + c], ps[:],
                                         AF.Identity, bias=bias[:, mt:mt + 1])
                for (o, c) in ncq:
                    pss = psum_mm([128, c])
                    nc.tensor.matmul(pss[:], Smat[:], qt[:, mt, o:o + c],
                                     start=True, stop=True)
                    t1 = TMPR.tile([128, 320], f32, tag='ropet1')
                    nc.vector.tensor_tensor(t1[:, 0:c], qt[:, mt, o:o + c],
                                            cos_s[:, mt, o:o + c], ALU.mult)
                    t2 = TMPR.tile([128, 320], f32, tag='ropet2')
                    nc.vector.tensor_tensor(t2[:, 0:c], pss[:],
                                            sin_s[:, mt, o:o + c], ALU.mult)
                    nc.vector.tensor_tensor(qt[:, mt, o:o + c], t1[:, 0:c],
                                            t2[:, 0:c], ALU.add)
            maybe_tap(f'{nm}_qrot', qt[:])

            # k projection + in-place rope
            if cross:
                ksrc, ctxT, ctx_xyz, krot, va = (
                    ctx_res['ctxT'], ctx_res['ctxT'], ctx_res['ctx_xyz'],
                    ctx_res['ctx_k'], ctx_res['ctx_va'])
            else:
                ksrc = TMP.tile([128, NT, T], bf16, tag='xkbf', name='xkbf')
                for kt in range(NT):
                    nc.vector.tensor_copy(ksrc[:, kt, :], x_resid[:, kt, :])
                krot = TMP.tile([128, NTP, T], bf16, tag='krot')
                va = TMP.tile([128, len(kvc), H, 65], bf16, tag='va')
            for mt in range(nkt):
                for (o, c) in nckv:
                    ps = psum_mm([128, c])
                    mmacc(ps[:], [(wk[:, kt, mt * 128:(mt + 1) * 128],
                                   ksrc[:, kt, o:o + c]) for kt in range(NT)])
                    nc.scalar.activation(krot[:, mt, o:o + c], ps[:],
                                         AF.Identity, bias=bias[:, 4 + mt:5 + mt])
                for (o, c) in nckv:
                    if cross:
                        psa = psum_mm([128, c])
                        nc.tensor.matmul(psa[:],
                                         divp[:, mt * 128:(mt + 1) * 128],
                                         ctx_xyz[:, o:o + c],
                                         start=True, stop=True)
                        cosk = ctx_res['CROPE'].tile([128, 320], f32,
                                                     tag='cosk', name='cosk')
                        nc.vector.add_range_wrap(cosk[:, 0:c], psa[:],
                                                 float(np.pi / 2), PI, TWO_PI)
                        nc.scalar.activation(cosk[:, 0:c], cosk[:, 0:c],
                                             AF.Sin)
                        sink = ctx_res['CROPE'].tile([128, 320], f32,
                                                     tag='sink', name='sink')
                        nc.vector.add_range_wrap(sink[:, 0:c], psa[:],
                                                 0.0, PI, TWO_PI)
                        nc.scalar.activation(sink[:, 0:c], sink[:, 0:c],
                                             AF.Sin)
                        cos_ap, sin_ap = cosk[:, 0:c], sink[:, 0:c]
                    else:
                        cos_ap = cos_s[:, mt, o:o + c]
                        sin_ap = sin_s[:, mt, o:o + c]
                    pss = psum_mm([128, c])
                    nc.tensor.matmul(pss[:], Smat[:], krot[:, mt, o:o + c],
                                     start=True, stop=True)
                    t1 = TMPR.tile([128, 320], f32, tag='ropet1')
                    nc.vector.tensor_tensor(t1[:, 0:c], krot[:, mt, o:o + c],
                                            cos_ap, ALU.mult)
                    t2 = TMPR.tile([128, 320], f32, tag='ropet2')
                    nc.vector.tensor_tensor(t2[:, 0:c], pss[:], sin_ap,
                                            ALU.mult)
                    nc.vector.tensor_tensor(krot[:, mt, o:o + c], t1[:, 0:c],
                                            t2[:, 0:c], ALU.add)
            maybe_tap(f'{nm}_krot', krot[:])

            # v projection (token-major) + ones column at 64
            nc.vector.memset(va[:, :, :, HD:65], 1.0)
            for ci, (o, c) in enumerate(kvc):
                ps = psum_mm([128, D])
                pairs = [(ksrc[:, kt, o:o + c], wv[:, kt, :])
                         for kt in range(NT)]
                pairs.append(((ones_r if cross else ones_r_bf)[:, 0:c],
                              bv[:]))
                mmacc(ps[0:c, :], pairs)
                nc.scalar.activation(
                    va[0:c, ci, :, 0:HD],
                    ps[0:c, :].rearrange('p (h d) -> p h d', h=H), AF.Copy)

            # attention (attout in padded head layout; pad rows are garbage
            # but multiply against zero rows of the padded Wo)
            attout = TMP.tile([128, NTP, T], bf16, tag='gp1', name='attout')
            nc.vector.memset(attout[32:64, :, :], 0.0)
            nc.vector.memset(attout[96:128, :, :], 0.0)
            for h in range(H):
                mt_q, off_q = h // 2, (h % 2) * 64
                for (oq, cq) in ncq:
                    Pb = PBP.tile([128, len(kvc), cq], pdt, tag='P')
                    for ci, (o, c) in enumerate(kvc):
                        ps = psum_mm([128, cq])
                        pairs = [(krot[off_q:off_q + HD, mt_q, o:o + c],
                                  qt[off_q:off_q + HD, mt_q, oq:oq + cq])]
                        mmacc(ps[0:c, :], pairs)
                        nc.scalar.activation(Pb[0:c, ci, 0:cq], ps[0:c, :],
                                             AF.Exp)
                    pav = psum_sm([65, cq])
                    mmacc(pav[:], [(va[0:c, ci, h, :], Pb[0:c, ci, 0:cq])
                                   for ci, (o, c) in enumerate(kvc)])
                    inv = TMPR.tile([1, 288], f32r, tag='inv')
                    nc.vector.reciprocal(inv[:, 0:cq], pav[64:65, :])
                    pb = psum_sm([HD, cq])
                    nc.tensor.matmul(pb[:], ones_r[:, 0:HD], inv[:, 0:cq],
                                     start=True, stop=True)
                    nc.scalar.activation(
                        attout[off_q:off_q + HD, mt_q, oq:oq + cq],
                        pav[0:HD, :], AF.Copy)
                    nc.vector.tensor_tensor(
                        attout[off_q:off_q + HD, mt_q, oq:oq + cq],
                        attout[off_q:off_q + HD, mt_q, oq:oq + cq],
                        pb[:], ALU.mult)
            maybe_tap(f'{nm}_attout', attout[:])

            # out projection + bias + residual
            x1 = TMP.tile([128, NT, T], f32r, tag='gp2')
            for mt in range(NT):
                for (o, c) in ncq:
                    ps = psum_mm([128, c])
                    mmacc(ps[:], [(wo[:, kt, mt * 128:(mt + 1) * 128],
                                   attout[:, kt, o:o + c])
                                  for kt in range(NTP)])
                    nc.vector.scalar_tensor_tensor(
                        x1[:, mt, o:o + c], ps[:], bias[:, 8 + mt:9 + mt],
                        x_resid[:, mt, o:o + c], ALU.add, ALU.add)

            xl = TMP.tile([128, NT, T], f32r, tag='xl')
            layer_norm(x1, xl, bias, 29, T, TMP)
            maybe_tap(f'{nm}_xl', xl[:])

            x2 = TMP.tile([128, NT, T], bf16, tag='gp1')
            for kt in range(NT):
                nc.scalar.activation(x2[:, kt, :], xl[:, kt, :], AF.Identity,
                                     bias=mods[:, 9 + kt:10 + kt],
                                     scale=s1f[:, kt:kt + 1])
            hbuf = TMP.tile([128, NT, T], bf16, tag='gp2')
            for mt in range(NT):
                for (o, c) in ncq:
                    ps = psum_mm([128, c])
                    mmacc(ps[:], [(w1[:, kt, mt * 128:(mt + 1) * 128],
                                   x2[:, kt, o:o + c]) for kt in range(NT)])
                    nc.scalar.activation(hbuf[:, mt, o:o + c], ps[:], AF.Relu,
                                         bias=bias[:, 11 + mt:12 + mt])
            x3 = TMP.tile([128, NT, T], f32r, tag='gp1')
            for mt in range(NT):
                for (o, c) in ncq:
                    ps = psum_mm([128, c])
                    mmacc(ps[:], [(w2[:, kt, mt * 128:(mt + 1) * 128],
                                   hbuf[:, kt, o:o + c]) for kt in range(NT)])
                    nc.vector.scalar_tensor_tensor(
                        x3[:, mt, o:o + c], ps[:], bias[:, 14 + mt:15 + mt],
                        xl[:, mt, o:o + c], ALU.add, ALU.add)
            layer_norm(x3, x_out, bias, 35, T, TMP)
            maybe_tap(f'{nm}_out', x_out[:, :, 0:T])

        # ---------------- cross phase ----------------
        with tc.tile_pool(name='wcross', bufs=1) as WPC, \
             tc.tile_pool(name='tmpc', bufs=1) as TMPC, \
             tc.tile_pool(name='crope', bufs=2) as CROPE, \
             tc.tile_pool(name='ctxp', bufs=1) as CP:
            ctx_res = {
                'CROPE': CROPE,
                'ctxT': CP.tile([128, NT, N], f32r, tag='ctxT',
                                name='ctxT'),
                'ctx_xyz': CP.tile([3, N], f32r, tag='ctx_xyz',
                                   name='ctx_xyz'),
                'ctx_k': CP.tile([128, NTP, N], bf16, tag='ctx_k',
                                 name='ctx_k'),
                'ctx_va': CP.tile([128, N // 128, H, 65], bf16,
                                  tag='ctx_va', name='ctx_va'),
            }
            nc.sync.dma_start(ctx_res['ctxT'][:],
                              dram['ctxT'].rearrange('a p f -> p a f'))
            nc.sync.dma_start(ctx_res['ctx_xyz'][:], dram['ctx_xyz'][:])
            attn_block('c0', residA[:, :, 0:L], residB[:, :, 0:L], L,
                       True, WPC, TMPC, ctx_res)
            attn_block('c1', residB[:, :, 0:L], residA[:, :, 0:L], L,
                       True, WPC, TMPC, ctx_res)

        # feats assembly: cross output already in residA cols 0:L
        nc.sync.dma_start(residA[:, :, L:TS],
                          dram['fpsT'].rearrange('a p f -> p a f'))
        maybe_tap('feats', residA[:])

        # ---------------- self phase + heads ----------------
        with tc.tile_pool(name='wself', bufs=2) as WPS, \
             tc.tile_pool(name='tmps', bufs=1) as TMPS:
            cur, nxt = residA, residB
            for nm in ('s0', 's1', 's2', 's3', 'p0', 'p1'):
                attn_block(nm, cur, nxt, TS, False, WPS, TMPS)
                cur, nxt = nxt, cur

            head_b = P.tile([128, 9], f32, tag='head_bias')
            nc.sync.dma_start(head_b[:], dram['head_bias'][:])
            posf = cur

            hw_tags = {'pproj': 'wv', 'pp1': 'wo', 'op1': 'w1'}

            def head_proj(wname, boff, src, func, Tsrc):
                wt = load_w(WPS, wname, hw_tags[wname])
                out = TMPS.tile([128, NT, L], f32r, tag=f'h_{wname}')
                for mt in range(NT):
                    ps = psum_mm([128, L])
                    mmacc(ps[:], [(wt[:, kt, mt * 128:(mt + 1) * 128],
                                   src[:, kt, 0:L]) for kt in range(NT)])
                    nc.scalar.activation(out[:, mt, :], ps[:], func,
                                         bias=head_b[:, boff + mt:boff + mt + 1])
                return out

            pf = head_proj('pproj', 0, posf, AF.Identity, TS)
            maybe_tap('posf', pf[:])
            h1 = head_proj('pp1', 3, pf, AF.Relu, L)
            h2 = head_proj('op1', 6, pf, AF.Relu, L)
            pp2T = load_w(WPS, 'pp2T', 'w2')
            op2T = load_w(WPS, 'op2T', 'wk')
            pp2b = load_w(WPS, 'pp2_b', 'bv')
            op2b = load_w(WPS, 'op2_b', 'bv')
            outsb = TMPS.tile([L, 8], f32, tag='outsb')
            ps = psum_sm([L, 7])
            mmacc32(ps[:], [(h1[:, kt, :], pp2T[:, kt, :])
                            for kt in range(NT)]
                    + [(ones_r[:, 0:L], pp2b[:])])
            nc.scalar.copy(outsb[:, 0:7], ps[:])
            ps2 = psum_sm([L, 1])
            mmacc32(ps2[:], [(h2[:, kt, :], op2T[:, kt, :])
                             for kt in range(NT)]
                     + [(ones_r[:, 0:L], op2b[:])])
            nc.scalar.copy(outsb[:, 7:8], ps2[:])
            nc.sync.dma_start(dram['out'][:], outsb[:])

    nc.compile()
    bd.dram = dram
    return bd


# ------------------------------------------------------------------- entry
_CACHE = {}


def kernel(**inputs):
    key = 'bd' + ','.join(sorted(DEBUG_TAPS))
    if key not in _CACHE:
        _CACHE[key] = build(tuple(DEBUG_TAPS))
    bd = _CACHE[key]
    w = prep_weights(inputs['params'])
    in_maps = []
    for b in range(N_CORES):
        m = dict(w)
        m.update(prep_core_inputs(b, inputs))
        in_maps.append(m)
    res = run_bass_kernel_spmd(bd.nc, in_maps, core_ids=list(range(N_CORES)))
    _CACHE['last_results'] = res
    out = np.stack([res.results[b]['out'] for b in range(N_CORES)], axis=0)
    return out.astype(np.float32)


# revision 31
# speedup vs baseline: 1.1198x; 1.0165x over previous
"""Trainium2 Bass kernel for nn_DiffuserJointer (dense diffusion transformer).

Strategy: data-parallel over batch B=8 across 8 NeuronCores (one batch
element per core; no collectives). On-core layout is feature-major
(features on partitions, tokens on the free dim) so every matmul contracts
over partitions with no transposes. All matmuls run in float32r (fp32 bits,
~tf32 precision, bf16-rate on the PE). Attention uses a transposed-scores
formulation (kv on partitions) so softmax normalization folds into an
augmented ones-column of V; RoPE's pair-shuffle is a constant 128x128
matmul; per-token normalizers broadcast via rank-1 matmuls.
"""
import sys
sys.path.insert(0, '/opt/trn_rl_repo')

import numpy as np
import ml_dtypes

import concourse.bass as bass
import concourse.bacc as bacc
import concourse.mybir as mybir
import concourse.tile as tile
from concourse.bass_utils import run_bass_kernel_spmd

f32 = mybir.dt.float32
f32r = mybir.dt.float32r
bf16 = mybir.dt.bfloat16
AF = mybir.ActivationFunctionType
ALU = mybir.AluOpType

# ---- problem dims (hardcoded) ----
D, H, B, L, N, K, NHIST = 384, 8, 8, 64, 2560, 512, 3
HD = D // H            # 48
DP = 512               # padded q/k feature dim (64 per head)
TS = L + K             # 576 self-attn tokens
NT = 3                 # feature tiles of 128
NTP = 4                # padded feature tiles
N_CORES = 8
EPS = 1e-5

# optional debug taps: list of names filled by the test harness before build
DEBUG_TAPS = []
_TAP_SHAPES = {}


# ---------------------------------------------------------------- host prep
def _sin_emb_table(x, dim):
    half = dim // 2
    freqs = np.exp(-np.log(10000.0) / (half - 1) * np.arange(half, dtype=np.float64))
    ang = np.asarray(x, np.float64)[:, None] * freqs[None, :]
    return np.concatenate([np.sin(ang), np.cos(ang)], axis=-1).astype(np.float32)


def _rope_div():
    d = D // 3  # 128
    return np.exp(np.arange(0, d, 2, dtype=np.float64)
                  * (-np.log(10000.0) / d)).astype(np.float32)


def _pad_rows(w):
    """(384, ...) head rows -> (512, ...): head h rows h*48..+47 -> h*64..+47."""
    out = np.zeros((DP,) + w.shape[1:], dtype=np.float32)
    for h in range(H):
        out[h * 64:h * 64 + HD] = w[h * HD:(h + 1) * HD]
    return out


def _tile_lhsT(wT):
    din = wT.shape[0]
    assert din % 128 == 0
    return np.ascontiguousarray(wT.reshape(din // 128, 128, *wT.shape[1:]))


def _cols(v):
    return np.ascontiguousarray(np.asarray(v, np.float32).reshape(NT, 128).T)


def _cols_p(v):
    return np.ascontiguousarray(np.asarray(v, np.float32).reshape(NTP, 128).T)


def _shuffle_mat(padded):
    S = np.zeros((128, 128), dtype=np.float32)
    if padded:
        for b0 in (0, 64):
            for i in range(0, HD, 2):
                S[b0 + i, b0 + i + 1] = -1.0
                S[b0 + i + 1, b0 + i] = 1.0
    else:
        for i in range(0, 128, 2):
            S[i, i + 1] = -1.0
            S[i + 1, i] = 1.0
    return np.ascontiguousarray(S.T)


def _divmat(padded):
    div = _rope_div()
    n = DP if padded else D
    M = np.zeros((3, n), dtype=np.float32)
    for r in range(n):
        if padded:
            h, j = r // 64, r % 64
            if j >= HD:
                continue
            d = h * HD + j
        else:
            d = r
        M[d // 128, r] = div[(d % 128) // 2]
    return M


def _pad_feat_T(x):
    """(T, 384) -> (512, T) padded feature-major."""
    xT = np.zeros((DP, x.shape[0]), dtype=np.float32)
    for h in range(H):
        xT[h * 64:h * 64 + HD] = x[:, h * HD:(h + 1) * HD].T
    return np.ascontiguousarray(xT)


def prep_weights(params):
    p = {k: (np.asarray(v, np.float32) if not isinstance(v, (list, dict)) else v)
         for k, v in params.items()}
    w = {}

    def blk(name, bp, cross):
        bp = {k: np.asarray(v, np.float32) for k, v in bp.items()}
        wq, wk, wv = bp['in_w'][:D], bp['in_w'][D:2 * D], bp['in_w'][2 * D:]
        bq, bk, bv = bp['in_b'][:D], bp['in_b'][D:2 * D], bp['in_b'][2 * D:]
        sc = HD ** -0.5
        bf = ml_dtypes.bfloat16
        wq_p, bq_p = _pad_rows(wq * sc), _pad_rows(bq * sc)
        w[f'{name}_wq'] = _tile_lhsT(wq_p.T.copy()).astype(bf)
        wk_t = _tile_lhsT(_pad_rows(wk).T.copy())
        w[f'{name}_wk'] = wk_t if cross else wk_t.astype(bf)
        bk_cols = _cols_p(_pad_rows(bk))
        wv_t = _tile_lhsT(wv.T.copy())
        w[f'{name}_wv'] = wv_t if cross else wv_t.astype(bf)
        w[f'{name}_wo'] = _tile_lhsT(_pad_rows(bp['out_w'].T.copy())).astype(bf)
        w[f'{name}_w1'] = _tile_lhsT(bp['w1'].T.copy()).astype(bf)
        w[f'{name}_w2'] = _tile_lhsT(bp['w2'].T.copy()).astype(bf)
        ada_cat = np.concatenate([bp['ada_w'], bp['f_ada_w']], axis=0)
        w[f'{name}_ada'] = _tile_lhsT(ada_cat.T.copy())       # (3,128,1536)
        cols = [
            _cols_p(bq_p),                               # 0:4
            bk_cols,                                     # 4:8
            _cols(bp['out_b']),                          # 8:11
            _cols(bp['b1']),                             # 11:14
            _cols(bp['b2']),                             # 14:17
            np.ascontiguousarray(np.concatenate(
                [bp['ada_b'], bp['f_ada_b']]).reshape(12, 128).T),  # 17:29
            _cols(bp['ln1_g']), _cols(bp['ln1_b']),      # 29:32, 32:35
            _cols(bp['ln2_g']), _cols(bp['ln2_b']),      # 35:38, 38:41
        ]
        w[f'{name}_bias'] = np.ascontiguousarray(np.concatenate(cols, axis=1))
        bv_t = np.ascontiguousarray(bv.reshape(1, D))
        w[f'{name}_bv'] = bv_t if cross else bv_t.astype(bf)
        w[f'{name}_adab'] = np.ascontiguousarray(np.concatenate(
            [bp['ada_b'], bp['f_ada_b']]).reshape(12, 128).T)

    for i, bp in enumerate(p['cross']):
        blk(f'c{i}', bp, cross=True)
    for i, bp in enumerate(p['selfa']):
        blk(f's{i}', bp, cross=False)
    for i, bp in enumerate(p['posa']):
        blk(f'p{i}', bp, cross=False)

    traj_aug = np.concatenate([p['traj_w'], p['traj_b'][:, None]], axis=1)
    w['traj_wT'] = np.ascontiguousarray(traj_aug.T)     # (8, 384)
    w['te1'] = _tile_lhsT(p['te1_w'].T.copy())
    w['te2'] = _tile_lhsT(p['te2_w'].T.copy())
    w['cg1'] = _tile_lhsT(p['cg1_w'].T.copy())          # (9,128,384)
    w['cg2'] = _tile_lhsT(p['cg2_w'].T.copy())
    w['enc_bias'] = np.ascontiguousarray(np.concatenate(
        [_cols(p['te1_b']), _cols(p['te2_b']), _cols(p['cg1_b']),
         _cols(p['cg2_b'])], axis=1))                   # (128, 12)
    w['pproj'] = _tile_lhsT(p['pproj_w'].T.copy())
    w['pp1'] = _tile_lhsT(p['pp1_w'].T.copy())
    w['op1'] = _tile_lhsT(p['op1_w'].T.copy())
    w['head_bias'] = np.ascontiguousarray(np.concatenate(
        [_cols(p['pproj_b']), _cols(p['pp1_b']), _cols(p['op1_b'])], axis=1))
    w['pp2T'] = _tile_lhsT(p['pp2_w'].T.copy())         # (3,128,7)
    w['op2T'] = _tile_lhsT(p['op2_w'].T.copy())         # (3,128,1)
    w['pp2_b'] = np.ascontiguousarray(p['pp2_b'].reshape(1, 7))
    w['op2_b'] = np.ascontiguousarray(p['op2_b'].reshape(1, 1))

    w['S_pad'] = _shuffle_mat(True)
    w['S_nop'] = _shuffle_mat(False)
    w['divmat_p'] = _divmat(True)                        # (3,512)
    w['divmat_n'] = _divmat(False)                       # (3,384)
    w['sinembL'] = _tile_lhsT(np.ascontiguousarray(
        _sin_emb_table(np.arange(L), D).T))              # (3,128,64)
    half = D // 2
    tf = np.exp(-np.log(10000.0) / (half - 1)
                * np.arange(half)).astype(np.float32)
    w['tfreq'] = np.ascontiguousarray(np.concatenate([tf, tf]).reshape(1, D))
    phase = np.zeros(D, np.float32)
    phase[half:] = np.pi / 2
    w['tphase'] = _cols(phase)                           # (128,3)
    return w


def prep_core_inputs(b, inputs):
    m = {}
    traj = np.asarray(inputs['trajectory'], np.float32)[b]          # (64,7)
    m['traj_aug'] = np.ascontiguousarray(
        np.concatenate([traj.T, np.ones((1, L), np.float32)], axis=0))
    m['traj_xyz'] = np.ascontiguousarray(traj[:, :3].T)             # (3,64)
    m['tstep'] = np.array([[float(np.asarray(inputs['timestep'])[b])]],
                          np.float32)
    m['ctxT'] = np.ascontiguousarray(
        np.asarray(inputs['context_feats'], np.float32)[b].T
        .reshape(NT, 128, N))
    m['ctx_xyz'] = np.ascontiguousarray(
        np.asarray(inputs['context'], np.float32)[b].T)             # (3,2560)
    m['gflat'] = np.ascontiguousarray(
        np.asarray(inputs['adaln_gripper_feats'], np.float32)[b]
        .reshape(-1).reshape(9, 128, 1))
    m['fpsT'] = np.ascontiguousarray(
        np.asarray(inputs['fps_feats'], np.float32)[:, b, :].T
        .reshape(NT, 128, K))
    fp = np.asarray(inputs['fps_pos'], np.float32)[b]               # (512,384,2)
    m['fps_cosT'] = np.ascontiguousarray(
        _pad_feat_T(fp[:, :, 0]).reshape(NTP, 128, K))
    m['fps_sinT'] = np.ascontiguousarray(
        _pad_feat_T(fp[:, :, 1]).reshape(NTP, 128, K))
    return m


# ------------------------------------------------------------- bass builder
def _nc_chunks(T):
    if T <= 512:
        return [(0, T)]
    if T == TS:
        return [(0, 288), (288, 288)]
    return [(i, 320) for i in range(0, T, 320)]


def _kv_chunks(T):
    out, i = [], 0
    while i < T:
        c = min(128, T - i)
        out.append((i, c))
        i += c
    return out


def _head_rows(h, padded):
    if padded:
        return [(h // 2, (h % 2) * 64, HD)]
    lo, hi = h * HD, (h + 1) * HD
    out = []
    for t in range(lo // 128, (hi - 1) // 128 + 1):
        s = max(lo, t * 128) - t * 128
        e = min(hi, (t + 1) * 128) - t * 128
        out.append((t, s, e - s))
    return out


class _BD:
    pass


def build(debug_taps=()):
    bd = _BD()
    nc = bacc.Bacc('TRN2', target_bir_lowering=False, debug=False,
                   num_devices=N_CORES)
    bd.nc = nc
    dram = {}

    def din(name, shape, dtype=f32r):
        dram[name] = nc.dram_tensor(name, list(shape), dtype,
                                    kind='ExternalInput').ap()

    def dout(name, shape, dtype=f32):
        dram[name] = nc.dram_tensor(name, list(shape), dtype,
                                    kind='ExternalOutput').ap()

    din('traj_aug', (8, L))
    din('traj_xyz', (3, L))
    din('tstep', (1, 1))
    din('ctxT', (NT, 128, N))
    din('ctx_xyz', (3, N))
    din('gflat', (9, 128, 1))
    din('fpsT', (NT, 128, K))
    din('fps_cosT', (NTP, 128, K))
    din('fps_sinT', (NTP, 128, K))
    blocks = ['c0', 'c1', 's0', 's1', 's2', 's3', 'p0', 'p1']
    for nm in blocks:
        cross = nm.startswith('c')
        din(f'{nm}_wq', (NT, 128, DP), bf16)
        din(f'{nm}_wk', (NT, 128, DP), f32r if cross else bf16)
        din(f'{nm}_wv', (NT, 128, D), f32r if cross else bf16)
        din(f'{nm}_wo', (NTP, 128, D), bf16)
        din(f'{nm}_w1', (NT, 128, D), bf16)
        din(f'{nm}_w2', (NT, 128, D), bf16)
        din(f'{nm}_ada', (NT, 128, 4 * D))
        din(f'{nm}_bias', (128, 41), f32)
        din(f'{nm}_adab', (128, 12), f32)
        din(f'{nm}_bv', (1, D), f32r if cross else bf16)
    din('traj_wT', (8, D))
    for nm2 in ('te1', 'te2', 'cg2', 'pproj', 'pp1', 'op1'):
        din(nm2, (NT, 128, D))
    din('cg1', (9, 128, D))
    din('enc_bias', (128, 12), f32)
    din('head_bias', (128, 9), f32)
    din('pp2T', (NT, 128, 7))
    din('op2T', (NT, 128, 1))
    din('pp2_b', (1, 7))
    din('op2_b', (1, 1))
    din('S_pad', (128, 128))
    din('S_nop', (128, 128))
    din('divmat_p', (3, DP))
    din('divmat_n', (3, D))
    din('sinembL', (NT, 128, L), f32)
    din('tfreq', (1, D))
    din('tphase', (128, 3), f32)
    dout('out', (L, 8))
    for tname in debug_taps:
        dout(f'tap_{tname}', _TAP_SHAPES[tname])

    def maybe_tap(name, ap):
        if name in debug_taps:
            nc.sync.dma_start(dram[f'tap_{name}'][:].bitcast(ap.dtype), ap)

    with tile.TileContext(nc) as tc, \
         nc.allow_low_precision(reason='f32r is full fp32 bits'), \
         tc.tile_pool(name='persist', bufs=1) as P, \
         tc.tile_pool(name='psA', bufs=5, space='PSUM') as PSA, \
         tc.tile_pool(name='psB', bufs=3, space='PSUM') as PSB, \
         tc.tile_pool(name='roptmp', bufs=2) as TMPR, \
         tc.tile_pool(name='pbuf', bufs=3) as PBP, \
         tc.tile_pool(name='lnt', bufs=3) as LNT:

        cnt = [0]

        def psum_mm(shape):
            cnt[0] += 1
            return PSA.tile(shape, f32, tag='mm', name=f'mm{cnt[0]}')

        def psum_sm(shape):
            cnt[0] += 1
            return PSB.tile(shape, f32, tag='sm', name=f'sm{cnt[0]}')

        def mmacc(ps, pairs):
            n = len(pairs)
            for i, (lhsT, rhs) in enumerate(pairs):
                nc.tensor.matmul(ps, lhsT, rhs, start=(i == 0),
                                 stop=(i == n - 1))

        def f32cast(ap):
            return ap.bitcast(f32) if ap.dtype == f32r else ap

        def mmacc32(ps, pairs):
            n = len(pairs)
            for i, (lhsT, rhs) in enumerate(pairs):
                nc.tensor.matmul(ps, f32cast(lhsT), f32cast(rhs),
                                 start=(i == 0), stop=(i == n - 1))

        def load_w(pool, name, tag):
            d = dram[name]
            sh = list(d.tensor.shape)
            if len(sh) == 3:
                t = pool.tile([128, sh[0], sh[2]], d.tensor.dtype, tag=tag)
                nc.sync.dma_start(t[:], d.rearrange('a p f -> p a f'))
            else:
                t = pool.tile(sh, d.tensor.dtype, tag=tag)
                nc.sync.dma_start(t[:], d[:])
            return t

        # ---------------- constants ----------------
        S_pad = P.tile([128, 128], f32r, tag='S_pad')
        nc.sync.dma_start(S_pad[:], dram['S_pad'][:])
        S_pad_bf = P.tile([128, 128], bf16, tag='S_pad_bf')
        nc.vector.tensor_copy(S_pad_bf[:], S_pad[:])
        ones_r_bf = P.tile([1, 128], bf16, tag='ones_r_bf')
        nc.vector.memset(ones_r_bf[:], 1.0)
        divp = P.tile([3, DP], f32r, tag='divp')
        nc.sync.dma_start(divp[:], dram['divmat_p'][:])
        sinembL = P.tile([128, NT, L], f32, tag='sinembL')
        nc.sync.dma_start(sinembL[:], dram['sinembL'].rearrange('a p f -> p a f'))
        tfreq = P.tile([1, D], f32r, tag='tfreq')
        nc.sync.dma_start(tfreq[:], dram['tfreq'][:])
        tphase = P.tile([128, 3], f32, tag='tphase')
        nc.sync.dma_start(tphase[:], dram['tphase'][:])
        ones_r = P.tile([1, 128], f32r, tag='ones_r')
        nc.vector.memset(ones_r[:].bitcast(f32), 1.0)
        ones_c = P.tile([128, 1], f32r, tag='ones_c')
        nc.vector.memset(ones_c[:].bitcast(f32), 1.0)
        halfpi = P.tile([128, 1], f32, tag='halfpi')
        nc.vector.memset(halfpi[:], float(np.pi / 2))
        epsc = P.tile([1, 1], f32, tag='epsc')
        nc.vector.memset(epsc[:], EPS)
        oneD_c = P.tile([128, 1], f32r, tag='oneD_c')
        nc.vector.memset(oneD_c[:].bitcast(f32), 1.0 / D)

        # ---------------- timestep/gripper embedding ----------------
        tstep = P.tile([1, 1], f32r, tag='tstep')
        nc.sync.dma_start(tstep[:], dram['tstep'][:])
        enc_b = P.tile([128, 12], f32, tag='enc_bias')
        nc.sync.dma_start(enc_b[:], dram['enc_bias'][:])
        tsin = P.tile([128, 3], f32r, tag='tsin')
        TWO_PI = float(2 * np.pi)
        PI = float(np.pi)
        C1 = 6.28125
        C2 = float(np.float32(2 * np.pi - C1))
        C3 = float(2 * np.pi - C1 - np.float32(2 * np.pi - C1))
        i32 = mybir.dt.int32
        for kt in range(NT):
            ps = psum_sm([128, 1])
            mmacc32(ps[:], [(tfreq[:, kt * 128:(kt + 1) * 128], tstep[:])])
            tv = TMPR.tile([128, 5], f32, tag='mcol', name='tv')
            ki = TMPR.tile([128, 1], i32, tag='mcoli', name='ki')
            xang, k4, kf, ycw, yw = (tv[:, 0:1], tv[:, 1:2], tv[:, 2:3],
                                     tv[:, 3:4], tv[:, 4:5])
            nc.vector.tensor_scalar_add(xang, ps[:], tphase[:, kt:kt + 1])
            nc.vector.tensor_scalar(k4, xang, 1.0 / TWO_PI, 0.5,
                                    ALU.mult, ALU.add)
            nc.vector.tensor_copy(ki[:], k4)
            nc.vector.tensor_copy(kf, ki[:])
            nc.vector.cody_waite_cascade(ycw, xang, kf, C1, C2, C3)
            nc.vector.add_range_wrap(yw, ycw, 0.0, PI, TWO_PI)
            nc.scalar.activation(tsin[:, kt:kt + 1], yw, AF.Sin)

        temb = P.tile([128, 3], f32r, tag='temb')
        st = P.tile([128, 3], f32r, tag='st')
        with tc.tile_pool(name='enc', bufs=1) as ENC:
            def vec_mlp(w1name, w2name, b_off, x_col, out_tile, nk=3):
                w1 = load_w(ENC, w1name, w1name)
                w2 = load_w(ENC, w2name, w2name)
                h = ENC.tile([128, 3], f32r, tag=f'h_{w1name}')
                for mt in range(NT):
                    ps = psum_sm([128, 1])
                    mmacc32(ps[:], [(w1[:, kk, mt * 128:(mt + 1) * 128],
                                     x_col[:, kk:kk + 1]) for kk in range(nk)])
                    nc.scalar.activation(h[:, mt:mt + 1], ps[:], AF.Relu,
                                         bias=enc_b[:, b_off + mt:b_off + mt + 1])
                for mt in range(NT):
                    ps = psum_sm([128, 1])
                    mmacc32(ps[:], [(w2[:, kk, mt * 128:(mt + 1) * 128],
                                     h[:, kk:kk + 1]) for kk in range(NT)])
                    nc.scalar.activation(out_tile[:, mt:mt + 1], ps[:],
                                         AF.Identity,
                                         bias=enc_b[:, b_off + 3 + mt:b_off + 4 + mt])

            tfe = ENC.tile([128, 3], f32r, tag='tfe')
            vec_mlp('te1', 'te2', 0, tsin, tfe)
            gflat = ENC.tile([128, 9], f32r, tag='gflat')
            nc.sync.dma_start(gflat[:],
                              dram['gflat'].rearrange('a p f -> p (a f)'))
            gfe = ENC.tile([128, 3], f32r, tag='gfe')
            vec_mlp('cg1', 'cg2', 6, gflat, gfe, nk=9)
            nc.vector.tensor_tensor(temb[:], tfe[:], gfe[:], ALU.add)
            sg = ENC.tile([128, 3], f32, tag='sg')
            nc.scalar.activation(sg[:], temb[:], AF.Sigmoid)
            nc.vector.tensor_tensor(st[:], temb[:], sg[:], ALU.mult)
        maybe_tap('temb', temb[:])

        ALL_BLOCKS = ['c0', 'c1', 's0', 's1', 's2', 's3', 'p0', 'p1']
        mods_all = P.tile([128, 12 * len(ALL_BLOCKS)], f32, tag='mods_all')
        with tc.tile_pool(name='adaw', bufs=2) as ADAW:
            for bi, nm in enumerate(ALL_BLOCKS):
                ada = load_w(ADAW, f'{nm}_ada', 'ada')
                adab = load_w(ADAW, f'{nm}_adab', 'adab')
                for mt in range(12):
                    ps = psum_sm([128, 1])
                    mmacc32(ps[:], [(ada[:, kk, mt * 128:(mt + 1) * 128],
                                     st[:, kk:kk + 1]) for kk in range(NT)])
                    nc.scalar.activation(
                        mods_all[:, bi * 12 + mt:bi * 12 + mt + 1], ps[:],
                        AF.Identity, bias=adab[:, mt:mt + 1])

        # ---------------- self/cross-q rope tables ----------------
        cos_s = P.tile([128, NTP, TS], f32, tag='cos_s')
        sin_s = P.tile([128, NTP, TS], f32, tag='sin_s')
        traj_xyz = P.tile([3, L], f32r, tag='traj_xyz')
        nc.sync.dma_start(traj_xyz[:], dram['traj_xyz'][:])
        for mt in range(NTP):
            ps = psum_mm([128, L])
            nc.tensor.matmul(ps[:], divp[:, mt * 128:(mt + 1) * 128],
                             traj_xyz[:], start=True, stop=True)
            for tab, ph in ((cos_s, float(np.pi / 2)), (sin_s, 0.0)):
                mt1 = TMPR.tile([128, 320], f32, tag='ropet1', name='mt1')
                nc.vector.add_range_wrap(mt1[:, 0:L], ps[:], ph, PI, TWO_PI)
                nc.scalar.activation(tab[:, mt, 0:L], mt1[:, 0:L], AF.Sin)
        nc.sync.dma_start(cos_s[:, :, L:TS],
                          dram['fps_cosT'].rearrange('a p f -> p a f')
                          .bitcast(f32))
        nc.sync.dma_start(sin_s[:, :, L:TS],
                          dram['fps_sinT'].rearrange('a p f -> p a f')
                          .bitcast(f32))

        # ---------------- trajectory encoder ----------------
        traj_aug = P.tile([8, L], f32r, tag='traj_aug')
        nc.sync.dma_start(traj_aug[:], dram['traj_aug'][:])
        traj_wT = P.tile([8, D], f32r, tag='traj_wT')
        nc.sync.dma_start(traj_wT[:], dram['traj_wT'][:])
        residA = P.tile([128, NT, TS], f32r, tag='residA')
        residB = P.tile([128, NT, TS], f32r, tag='residB')
        for mt in range(NT):
            ps = psum_mm([128, L])
            nc.tensor.matmul(ps[:], traj_wT[:, mt * 128:(mt + 1) * 128],
                             traj_aug[:], start=True, stop=True)
            nc.vector.tensor_tensor(residA[:, mt, 0:L], ps[:],
                                    sinembL[:, mt, :], ALU.add)
        maybe_tap('trajenc', residA[:, :, 0:L])

        # ---------------- shared block pieces ----------------
        def adaln_mods(nm, TMP):
            bi = ALL_BLOCKS.index(nm)
            mods = mods_all[:, bi * 12:(bi + 1) * 12]
            s1 = TMP.tile([128, 3], f32, tag='s1')
            nc.vector.tensor_scalar_add(s1[:], mods[:, 0:3], 1.0)
            s1f = TMP.tile([128, 3], f32, tag='s1f')
            nc.vector.tensor_scalar_add(s1f[:], mods[:, 6:9], 1.0)
            return mods, s1, s1f

        def layer_norm(x_in, x_out, bias, boff, T, TMP):
            ncs = _nc_chunks(T)
            work = TMP.tile([1, 2 * T], f32, tag='lnwork', name='lnwork')
            msq, var = work[:, 0:T], work[:, T:2 * T]
            rr = TMP.tile([1, 2 * T], f32r, tag='lnrr', name='lnrr')
            rstd, mr = rr[:, 0:T], rr[:, T:2 * T]
            for (o, c) in ncs:
                ps = psum_sm([1, c])
                mmacc(ps[:], [(oneD_c[:], x_in[:, kt, o:o + c])
                              for kt in range(NT)])
                ps2 = psum_sm([1, c])
                for kt in range(NT):
                    sqc = LNT.tile([128, 512], f32r, tag='lnsq', name='sqc')
                    nc.scalar.activation(sqc[:, 0:c], x_in[:, kt, o:o + c],
                                         AF.Square)
                    nc.tensor.matmul(ps2[:], oneD_c[:], sqc[:, 0:c],
                                     start=(kt == 0), stop=(kt == NT - 1))
                # msq = mean^2 ; var = ex2 - msq ; rstd = 1/sqrt(var+eps)
                nc.scalar.activation(msq[:, o:o + c], ps[:], AF.Square)
                nc.vector.scalar_tensor_tensor(var[:, o:o + c], ps2[:], 1.0,
                                               msq[:, o:o + c], ALU.mult,
                                               ALU.subtract)
                nc.scalar.activation(var[:, o:o + c], var[:, o:o + c],
                                     AF.Sqrt, bias=epsc[:])
                nc.vector.reciprocal(rstd[:, o:o + c], var[:, o:o + c])
                nc.vector.tensor_tensor(mr[:, o:o + c], ps[:],
                                        rstd[:, o:o + c], ALU.mult)
            for (o, c) in ncs:
                psr = psum_mm([128, c])
                nc.tensor.matmul(psr[:], ones_r[:], rstd[:, o:o + c],
                                 start=True, stop=True)
                psm = psum_mm([128, c])
                nc.tensor.matmul(psm[:], ones_r[:], mr[:, o:o + c],
                                 start=True, stop=True)
                for kt in range(NT):
                    t1 = LNT.tile([128, 512], f32, tag='lnt1', name='lnt1')
                    nc.vector.tensor_tensor(t1[:, 0:c], x_in[:, kt, o:o + c],
                                            psr[:], ALU.mult)
                    nc.vector.tensor_tensor(t1[:, 0:c], t1[:, 0:c], psm[:],
                                            ALU.subtract)
                    nc.scalar.activation(
                        x_out[:, kt, o:o + c], t1[:, 0:c], AF.Identity,
                        bias=bias[:, boff + 3 + kt:boff + 4 + kt],
                        scale=bias[:, boff + kt:boff + 1 + kt])

        def attn_block(nm, x_resid, x_out, T, cross, WPb, TMP,
                       ctx_res=None):
            Tkv = N if cross else T
            bias = load_w(WPb, f'{nm}_bias', 'bias')
            wq = load_w(WPb, f'{nm}_wq', 'wq')
            wk = load_w(WPb, f'{nm}_wk', 'wk')
            wv = load_w(WPb, f'{nm}_wv', 'wv')
            wo = load_w(WPb, f'{nm}_wo', 'wo')
            w1 = load_w(WPb, f'{nm}_w1', 'w1')
            w2 = load_w(WPb, f'{nm}_w2', 'w2')
            bv = load_w(WPb, f'{nm}_bv', 'bv')
            mods, s1, s1f = adaln_mods(nm, TMP)

            ncq = _nc_chunks(T)
            nckv = _nc_chunks(Tkv)
            kvc = _kv_chunks(Tkv)
            nkt = NTP
            pdt = bf16
            Smat = S_pad_bf

            # adaln on q input
            aq = TMP.tile([128, NT, T], bf16, tag='gp1')
            for kt in range(NT):
                nc.scalar.activation(aq[:, kt, :], x_resid[:, kt, :],
                                     AF.Identity, bias=mods[:, 3 + kt:4 + kt],
                                     scale=s1[:, kt:kt + 1])

            # q projection (padded) + in-place rope
            qt = TMP.tile([128, NTP, T], pdt, tag='qt')
            for mt in range(NTP):
                for (o, c) in ncq:
                    ps = psum_mm([128, c])
                    mmacc(ps[:], [(wq[:, kt, mt * 128:(mt + 1) * 128],
                                   aq[:, kt, o:o + c]) for kt in range(NT)])
                    nc.scalar.activation(qt[:, mt, o:o + c], ps[:],
                                         AF.Identity, bias=bias[:, mt:mt + 1])
                for (o, c) in ncq:
                    pss = psum_mm([128, c])
                    nc.tensor.matmul(pss[:], Smat[:], qt[:, mt, o:o + c],
                                     start=True, stop=True)
                    t1 = TMPR.tile([128, 320], f32, tag='ropet1')
                    nc.vector.tensor_tensor(t1[:, 0:c], qt[:, mt, o:o + c],
                                            cos_s[:, mt, o:o + c], ALU.mult)
                    t2 = TMPR.tile([128, 320], f32, tag='ropet2')
                    nc.vector.tensor_tensor(t2[:, 0:c], pss[:],
                                            sin_s[:, mt, o:o + c], ALU.mult)
                    nc.vector.tensor_tensor(qt[:, mt, o:o + c], t1[:, 0:c],
                                            t2[:, 0:c], ALU.add)
            maybe_tap(f'{nm}_qrot', qt[:])

            # k projection + in-place rope
            if cross:
                ksrc, ctxT, ctx_xyz, krot, va = (
                    ctx_res['ctxT'], ctx_res['ctxT'], ctx_res['ctx_xyz'],
                    ctx_res['ctx_k'], ctx_res['ctx_va'])
            else:
                ksrc = TMP.tile([128, NT, T], bf16, tag='xkbf', name='xkbf')
                for kt in range(NT):
                    nc.gpsimd.tensor_copy(ksrc[:, kt, :], x_resid[:, kt, :])
                krot = TMP.tile([128, NTP, T], bf16, tag='krot')
                va = TMP.tile([128, len(kvc), H, 65], bf16, tag='va')
            for mt in range(nkt):
                for (o, c) in nckv:
                    ps = psum_mm([128, c])
                    mmacc(ps[:], [(wk[:, kt, mt * 128:(mt + 1) * 128],
                                   ksrc[:, kt, o:o + c]) for kt in range(NT)])
                    nc.scalar.activation(krot[:, mt, o:o + c], ps[:],
                                         AF.Identity, bias=bias[:, 4 + mt:5 + mt])
                for (o, c) in nckv:
                    if cross:
                        psa = psum_mm([128, c])
                        nc.tensor.matmul(psa[:],
                                         divp[:, mt * 128:(mt + 1) * 128],
                                         ctx_xyz[:, o:o + c],
                                         start=True, stop=True)
                        cosk = ctx_res['CROPE'].tile([128, 320], f32,
                                                     tag='cosk', name='cosk')
                        nc.vector.add_range_wrap(cosk[:, 0:c], psa[:],
                                                 float(np.pi / 2), PI, TWO_PI)
                        nc.scalar.activation(cosk[:, 0:c], cosk[:, 0:c],
                                             AF.Sin)
                        sink = ctx_res['CROPE'].tile([128, 320], f32,
                                                     tag='sink', name='sink')
                        nc.vector.add_range_wrap(sink[:, 0:c], psa[:],
                                                 0.0, PI, TWO_PI)
                        nc.scalar.activation(sink[:, 0:c], sink[:, 0:c],
                                             AF.Sin)
                        cos_ap, sin_ap = cosk[:, 0:c], sink[:, 0:c]
                    else:
                        cos_ap = cos_s[:, mt, o:o + c]
                        sin_ap = sin_s[:, mt, o:o + c]
                    pss = psum_mm([128, c])
                    nc.tensor.matmul(pss[:], Smat[:], krot[:, mt, o:o + c],
                                     start=True, stop=True)
                    t1 = TMPR.tile([128, 320], f32, tag='ropet1')
                    nc.vector.tensor_tensor(t1[:, 0:c], krot[:, mt, o:o + c],
                                            cos_ap, ALU.mult)
                    t2 = TMPR.tile([128, 320], f32, tag='ropet2')
                    nc.vector.tensor_tensor(t2[:, 0:c], pss[:], sin_ap,
                                            ALU.mult)
                    nc.vector.tensor_tensor(krot[:, mt, o:o + c], t1[:, 0:c],
                                            t2[:, 0:c], ALU.add)
            maybe_tap(f'{nm}_krot', krot[:])

            # v projection (token-major) + ones column at 64
            nc.vector.memset(va[:, :, :, HD:65], 1.0)
            for ci, (o, c) in enumerate(kvc):
                ps = psum_mm([128, D])
                pairs = [(ksrc[:, kt, o:o + c], wv[:, kt, :])
                         for kt in range(NT)]
                pairs.append(((ones_r if cross else ones_r_bf)[:, 0:c],
                              bv[:]))
                mmacc(ps[0:c, :], pairs)
                nc.scalar.activation(
                    va[0:c, ci, :, 0:HD],
                    ps[0:c, :].rearrange('p (h d) -> p h d', h=H), AF.Copy)

            # attention (attout in padded head layout; pad rows are garbage
            # but multiply against zero rows of the padded Wo)
            attout = TMP.tile([128, NTP, T], bf16, tag='gp1', name='attout')
            nc.vector.memset(attout[32:64, :, :], 0.0)
            nc.vector.memset(attout[96:128, :, :], 0.0)
            for h in range(H):
                mt_q, off_q = h // 2, (h % 2) * 64
                for (oq, cq) in ncq:
                    Pb = PBP.tile([128, len(kvc), cq], pdt, tag='P')
                    for ci, (o, c) in enumerate(kvc):
                        ps = psum_mm([128, cq])
                        pairs = [(krot[off_q:off_q + HD, mt_q, o:o + c],
                                  qt[off_q:off_q + HD, mt_q, oq:oq + cq])]
                        mmacc(ps[0:c, :], pairs)
                        nc.scalar.activation(Pb[0:c, ci, 0:cq], ps[0:c, :],
                                             AF.Exp)
                    pav = psum_sm([65, cq])
                    mmacc(pav[:], [(va[0:c, ci, h, :], Pb[0:c, ci, 0:cq])
                                   for ci, (o, c) in enumerate(kvc)])
                    inv = TMPR.tile([1, 288], f32r, tag='inv')
                    nc.vector.reciprocal(inv[:, 0:cq], pav[64:65, :])
                    pb = psum_sm([HD, cq])
                    nc.tensor.matmul(pb[:], ones_r[:, 0:HD], inv[:, 0:cq],
                                     start=True, stop=True)
                    nc.scalar.activation(
                        attout[off_q:off_q + HD, mt_q, oq:oq + cq],
                        pav[0:HD, :], AF.Copy)
                    nc.vector.tensor_tensor(
                        attout[off_q:off_q + HD, mt_q, oq:oq + cq],
                        attout[off_q:off_q + HD, mt_q, oq:oq + cq],
                        pb[:], ALU.mult)
            maybe_tap(f'{nm}_attout', attout[:])

            # out projection + bias + residual
            x1 = TMP.tile([128, NT, T], f32r, tag='gp2')
            for mt in range(NT):
                for (o, c) in ncq:
                    ps = psum_mm([128, c])
                    mmacc(ps[:], [(wo[:, kt, mt * 128:(mt + 1) * 128],
                                   attout[:, kt, o:o + c])
                                  for kt in range(NTP)])
                    nc.vector.scalar_tensor_tensor(
                        x1[:, mt, o:o + c], ps[:], bias[:, 8 + mt:9 + mt],
                        x_resid[:, mt, o:o + c], ALU.add, ALU.add)

            xl = TMP.tile([128, NT, T], f32r, tag='xl')
            layer_norm(x1, xl, bias, 29, T, TMP)
            maybe_tap(f'{nm}_xl', xl[:])

            x2 = TMP.tile([128, NT, T], bf16, tag='gp1')
            for kt in range(NT):
                nc.scalar.activation(x2[:, kt, :], xl[:, kt, :], AF.Identity,
                                     bias=mods[:, 9 + kt:10 + kt],
                                     scale=s1f[:, kt:kt + 1])
            hbuf = TMP.tile([128, NT, T], bf16, tag='gp2')
            for mt in range(NT):
                for (o, c) in ncq:
                    ps = psum_mm([128, c])
                    mmacc(ps[:], [(w1[:, kt, mt * 128:(mt + 1) * 128],
                                   x2[:, kt, o:o + c]) for kt in range(NT)])
                    nc.scalar.activation(hbuf[:, mt, o:o + c], ps[:], AF.Relu,
                                         bias=bias[:, 11 + mt:12 + mt])
            x3 = TMP.tile([128, NT, T], f32r, tag='gp1')
            for mt in range(NT):
                for (o, c) in ncq:
                    ps = psum_mm([128, c])
                    mmacc(ps[:], [(w2[:, kt, mt * 128:(mt + 1) * 128],
                                   hbuf[:, kt, o:o + c]) for kt in range(NT)])
                    nc.vector.scalar_tensor_tensor(
                        x3[:, mt, o:o + c], ps[:], bias[:, 14 + mt:15 + mt],
                        xl[:, mt, o:o + c], ALU.add, ALU.add)
            layer_norm(x3, x_out, bias, 35, T, TMP)
            maybe_tap(f'{nm}_out', x_out[:, :, 0:T])

        # ---------------- cross phase ----------------
        with tc.tile_pool(name='wcross', bufs=1) as WPC, \
             tc.tile_pool(name='tmpc', bufs=1) as TMPC, \
             tc.tile_pool(name='crope', bufs=2) as CROPE, \
             tc.tile_pool(name='ctxp', bufs=1) as CP:
            ctx_res = {
                'CROPE': CROPE,
                'ctxT': CP.tile([128, NT, N], f32r, tag='ctxT',
                                name='ctxT'),
                'ctx_xyz': CP.tile([3, N], f32r, tag='ctx_xyz',
                                   name='ctx_xyz'),
                'ctx_k': CP.tile([128, NTP, N], bf16, tag='ctx_k',
                                 name='ctx_k'),
                'ctx_va': CP.tile([128, N // 128, H, 65], bf16,
                                  tag='ctx_va', name='ctx_va'),
            }
            nc.sync.dma_start(ctx_res['ctxT'][:],
                              dram['ctxT'].rearrange('a p f -> p a f'))
            nc.sync.dma_start(ctx_res['ctx_xyz'][:], dram['ctx_xyz'][:])
            attn_block('c0', residA[:, :, 0:L], residB[:, :, 0:L], L,
                       True, WPC, TMPC, ctx_res)
            attn_block('c1', residB[:, :, 0:L], residA[:, :, 0:L], L,
                       True, WPC, TMPC, ctx_res)

        # feats assembly: cross output already in residA cols 0:L
        nc.sync.dma_start(residA[:, :, L:TS],
                          dram['fpsT'].rearrange('a p f -> p a f'))
        maybe_tap('feats', residA[:])

        # ---------------- self phase + heads ----------------
        with tc.tile_pool(name='wself', bufs=2) as WPS, \
             tc.tile_pool(name='tmps', bufs=1) as TMPS:
            cur, nxt = residA, residB
            for nm in ('s0', 's1', 's2', 's3', 'p0', 'p1'):
                attn_block(nm, cur, nxt, TS, False, WPS, TMPS)
                cur, nxt = nxt, cur

            head_b = P.tile([128, 9], f32, tag='head_bias')
            nc.sync.dma_start(head_b[:], dram['head_bias'][:])
            posf = cur

            hw_tags = {'pproj': 'wv', 'pp1': 'wo', 'op1': 'w1'}

            def head_proj(wname, boff, src, func, Tsrc):
                wt = load_w(WPS, wname, hw_tags[wname])
                out = TMPS.tile([128, NT, L], f32r, tag=f'h_{wname}')
                for mt in range(NT):
                    ps = psum_mm([128, L])
                    mmacc(ps[:], [(wt[:, kt, mt * 128:(mt + 1) * 128],
                                   src[:, kt, 0:L]) for kt in range(NT)])
                    nc.scalar.activation(out[:, mt, :], ps[:], func,
                                         bias=head_b[:, boff + mt:boff + mt + 1])
                return out

            pf = head_proj('pproj', 0, posf, AF.Identity, TS)
            maybe_tap('posf', pf[:])
            h1 = head_proj('pp1', 3, pf, AF.Relu, L)
            h2 = head_proj('op1', 6, pf, AF.Relu, L)
            pp2T = load_w(WPS, 'pp2T', 'w2')
            op2T = load_w(WPS, 'op2T', 'wk')
            pp2b = load_w(WPS, 'pp2_b', 'bv')
            op2b = load_w(WPS, 'op2_b', 'bv')
            outsb = TMPS.tile([L, 8], f32, tag='outsb')
            ps = psum_sm([L, 7])
            mmacc32(ps[:], [(h1[:, kt, :], pp2T[:, kt, :])
                            for kt in range(NT)]
                    + [(ones_r[:, 0:L], pp2b[:])])
            nc.scalar.copy(outsb[:, 0:7], ps[:])
            ps2 = psum_sm([L, 1])
            mmacc32(ps2[:], [(h2[:, kt, :], op2T[:, kt, :])
                             for kt in range(NT)]
                     + [(ones_r[:, 0:L], op2b[:])])
            nc.scalar.copy(outsb[:, 7:8], ps2[:])
            nc.sync.dma_start(dram['out'][:], outsb[:])

    nc.compile()
    bd.dram = dram
    return bd


# ------------------------------------------------------------------- entry
_CACHE = {}


def kernel(**inputs):
    key = 'bd' + ','.join(sorted(DEBUG_TAPS))
    if key not in _CACHE:
        _CACHE[key] = build(tuple(DEBUG_TAPS))
    bd = _CACHE[key]
    w = prep_weights(inputs['params'])
    in_maps = []
    for b in range(N_CORES):
        m = dict(w)
        m.update(prep_core_inputs(b, inputs))
        in_maps.append(m)
    res = run_bass_kernel_spmd(bd.nc, in_maps, core_ids=list(range(N_CORES)))
    _CACHE['last_results'] = res
    out = np.stack([res.results[b]['out'] for b in range(N_CORES)], axis=0)
    return out.astype(np.float32)
